# revision 1
# baseline (speedup 1.0000x reference)
"""Trainium2 Bass kernel for nn_HardcodedKVMemoryBlock (8 NeuronCores).

Sharding: core i handles batch b=i//4, sequence chunk c=i%4 (512 tokens).
The (B,L,P,D) cumsum is restructured as causal linear attention:
    retrieved = tril(Q K^T) @ V + Q @ S_prefix
with Q=[cos,sin] phasors (L x 64), V = values at odd positions, and the
cross-chunk carry S_prefix = (K_even^T @ x_odd) @ val_W computed
redundantly per core from a zero-padded prefix (no collectives).
The 1/sqrt(valid*P) normalization cancels inside LayerNorm1 (scale
invariance); ln gains are folded into W1/Wo and means are applied as
rank-1 PE updates, so LN costs no extra full-width element-wise passes
beyond one multiply by the broadcast rstd.

v2: host supplies x^T and x_even_prefix^T (kills 20 PE transposes),
activation-table discipline via a dummy Silu (2 table loads instead of
5), cos via Sin(-pi*|t| + pi/2) (no DVE prep op), batched kpre^T
transpose copy, 1-iteration Newton rsqrt, and element-wise work spread
across scalar/vector/pool engines to keep the PE stream dense.
"""

import math
import numpy as np
import ml_dtypes

import concourse.bass as bass
import concourse.tile as tile
from concourse import bacc, mybir
from concourse.bass_utils import run_bass_kernel_spmd

PI = math.pi
B, L, D, P = 2, 2048, 256, 32
T = 512          # own tokens per core
H = 512          # MLP hidden
PRE = 768        # padded prefix pair count (max prefix 1536 tokens / 2)
N_CORES = 8

f32 = mybir.dt.float32
f32r = mybir.dt.float32r
bf16 = mybir.dt.bfloat16
AF = mybir.ActivationFunctionType
OP = mybir.AluOpType


def _r(ap):
    return ap.bitcast(f32r)


def _build():
    nc = bacc.Bacc("TRN2", target_bir_lowering=False, debug=False,
                   num_devices=N_CORES)

    def din(name, shape, dt):
        return nc.dram_tensor(name, shape, dt, kind="ExternalInput").ap()

    xc = din("xc", [T, D], f32)
    xt = din("xt", [D, T], f32r)         # x^T (host-transposed)
    xpet = din("xpet", [D, PRE], bf16)   # x_even_prefix^T (host-transposed)
    xpo = din("xpo", [PRE, D], bf16)
    kw = din("kw", [D, P], f32r)
    kwb = din("kwb", [D, P], bf16)
    kbc = din("kbc", [P, 1], f32)
    vw = din("vw", [D, D], f32r)
    w1g = din("w1g", [D, H], bf16)
    w2 = din("w2", [H, D], bf16)
    wog = din("wog", [D, D], bf16)
    cpack = din("cpack", [128, 384], f32)   # m0 | eyef
    cr1 = din("cr1", [1, H + D], bf16)      # c1n | c1on
    cc1 = din("cc1", [128, 6], f32)         # cb1 (4) | b2c (2)
    outc = nc.dram_tensor("outc", [T, D], f32, kind="ExternalOutput").ap()

    with tile.TileContext(nc) as tc:
        _emit(tc, locals())
    nc.compile()
    return nc


def _emit(tc, io):
    nc = tc.nc
    xc, xt, xpet, xpo = io["xc"], io["xt"], io["xpet"], io["xpo"]
    outc = io["outc"]

    sb = tc.alloc_tile_pool(name="sb", bufs=1)
    pt = tc.alloc_tile_pool(name="pt", bufs=2, space="PSUM")    # transposes
    pa = tc.alloc_tile_pool(name="pa", bufs=2, space="PSUM")    # narrow tiles
    pb = tc.alloc_tile_pool(name="pb", bufs=3, space="PSUM")    # [128,512]

    # ---------------- constant / weight tiles ----------------
    kw_sb = sb.tile([128, 2 * P], f32r)       # ktile kt at cols [P*kt]
    kwb_sb = sb.tile([128, 2 * P], bf16)
    kbc_sb = sb.tile([P, 1], f32)
    vw_sb = sb.tile([128, 512], f32r)         # ktile kt at cols [256*kt]
    w1g_sb = sb.tile([128, 1024], bf16)      # ktile kt at cols [512*kt]
    w2_sb = sb.tile([128, 1024], bf16)       # ktile kt at cols [256*kt]
    wog_sb = sb.tile([128, 512], bf16)       # ktile dh at cols [256*dh]
    cpack_sb = sb.tile([128, 384], f32)
    eyeb_sb = sb.tile([64, 64], bf16)
    ones_sb = sb.tile([1, 128], f32r)
    invd_sb = sb.tile([128, 1], f32r)
    invdb_sb = sb.tile([128, 1], bf16)
    cr1_sb = sb.tile([1, H + D], bf16)
    cc1_sb = sb.tile([128, 6], f32)
    halfpi_sb = sb.tile([P, 1], f32)
    zerop_sb = sb.tile([P, 1], f32)
    zero128_sb = sb.tile([128, 1], f32)
    zero1_sb = sb.tile([1, 1], f32)

    m0_sb = cpack_sb[:, 0:256]
    eyef_sb = cpack_sb[:, 256:384]
    c1n_sb = cr1_sb[:, 0:H]
    c1on_sb = cr1_sb[:, H:H + D]
    cb1_sb = cc1_sb[:, 0:4]
    b2c_sb = cc1_sb[:, 4:6]
    dma = nc.sync.dma_start
    dmaw = nc.gpsimd.dma_start
    nc.vector.memset(halfpi_sb[:], PI / 2)
    nc.vector.memset(zerop_sb[:], 0.0)
    nc.vector.memset(zero128_sb[:], 0.0)
    nc.vector.memset(zero1_sb[:], 0.0)
    nc.vector.memset(ones_sb[:].bitcast(f32), 1.0)
    nc.vector.memset(invd_sb[:].bitcast(f32), 1.0 / D)
    nc.vector.memset(invdb_sb[:], 1.0 / D)

    # ---------------- data tiles ----------------
    xc_sb = sb.tile([128, 1024], f32)        # token tile tt at cols [256*tt]
    xT_sb = sb.tile([128, 1024], f32r)        # d-half dh at cols [512*dh]
    xpeT_sb = sb.tile([128, 1536], bf16)     # ktile kt at cols [768*kt]
    xpo_sb = sb.tile([128, 1536], bf16)      # block j at cols [256*j]
    tv_sb = sb.tile([2 * P, T], f32)
    qb_sb = sb.tile([2 * P, T], f32r)         # rows 0:32 cos, 32:64 sin
    tvp_sb = sb.tile([2 * P, PRE], f32)
    kpre_sb = sb.tile([2 * P, PRE], bf16)
    kpreT_sb = sb.tile([128, 6 * 64], bf16)  # block j at cols [64*j]
    gT_sb = sb.tile([128, 128], f32r)         # ktile dh at cols [64*dh]
    s_sb = sb.tile([2 * P, D], f32r)
    vodd_sb = sb.tile([128, 512], f32r)       # block blk at cols [256*blk]
    ss0_sb = sb.tile([128, 512], f32r)
    ss1_sb = sb.tile([128, 256], f32r)
    cross_sb = sb.tile([128, 1024], f32)
    r_sb = sb.tile([128, 1024], bf16)         # retrieved, dh at cols [512*dh]
    sq_sb = sb.tile([128, 1024], bf16)
    h_sb = sb.tile([128, 2048], bf16)        # mtile m at cols [512*m]
    f_sb = sb.tile([128, 1024], bf16)         # refined
    sq2_sb = sb.tile([128, 1024], bf16)
    out_sb = sb.tile([128, 1024], f32)       # token tile tt at cols [256*tt]
    dum_sb = sb.tile([1, 1], f32)

    var1_sb = sb.tile([1, T], f32)
    rstd1_sb = sb.tile([1, T], f32r)
    mean1_sb = sb.tile([1, T], bf16)
    rb1s_sb = sb.tile([128, T], f32)
    hi_sb = sb.tile([128, 2048], bf16)
    m2_sb = sb.tile([1, T], f32)
    stdc_sb = sb.tile([128, 4], f32)
    rstdc_sb = sb.tile([128, 4], f32)
    var2_sb = sb.tile([1, T], f32)
    mean2_sb = sb.tile([1, T], bf16)
    m2b_sb = sb.tile([1, T], f32)
    stdc2_sb = sb.tile([128, 4], f32)
    rstdc2_sb = sb.tile([128, 4], f32)
    nwt1_sb = sb.tile([128, 4], f32)
    nwt2_sb = sb.tile([128, 4], f32)
    cmh_sb = sb.tile([128, 4], f32)
    c15_sb = sb.tile([128, 4], f32)
    nc.vector.memset(cmh_sb[:], -0.5)
    nc.vector.memset(c15_sb[:], 1.5)

    from concourse.tile import add_dep_helper
    # critical-path loads first; descriptor generation is serialized per
    # engine queue, so spread the early loads across three queues
    d_xt0 = dma(xT_sb[:, 0:512], xt[0:128, :])
    d_xt1 = dma(xT_sb[:, 512:1024], xt[128:256, :])
    dma(cpack_sb[:], io["cpack"])
    nc.scalar.dma_start(kw_sb[:], io["kw"].rearrange("(k p) q -> p k q", p=128))
    nc.scalar.dma_start(kwb_sb[:], io["kwb"].rearrange("(k p) q -> p k q", p=128))
    nc.scalar.dma_start(kbc_sb[:], io["kbc"])
    d_xpe0 = nc.scalar.dma_start(xpeT_sb[:, 0:768], xpet[0:128, :])
    d_xpe1 = nc.scalar.dma_start(xpeT_sb[:, 768:1536], xpet[128:256, :])
    nc.vector.tensor_copy(eyeb_sb[:], eyef_sb[0:64, 0:64])
    wd = []
    wd.append(dmaw(vw_sb[:], io["vw"].rearrange("(k p) q -> p k q", p=128)))
    wd.append(dmaw(xpo_sb[:], xpo.rearrange("(j p) d -> p j d", p=128)))
    wd.append(dmaw(w1g_sb[:], io["w1g"].rearrange("(k p) q -> p k q", p=128)))
    wd.append(dmaw(cr1_sb[:], io["cr1"]))
    wd.append(dmaw(cc1_sb[:], io["cc1"]))
    wd.append(dmaw(w2_sb[:], io["w2"].rearrange("(k p) q -> p k q", p=128)))
    wd.append(dmaw(wog_sb[:], io["wog"].rearrange("(k p) q -> p k q", p=128)))
    wd.append(dmaw(xc_sb[:], xc.rearrange("(t p) d -> p t d", p=128)))
    # bulk weights wait for the latency-critical input loads to finish so
    # they don't steal HBM bandwidth from the critical path
    for w in wd:
        add_dep_helper(w.ins, d_xpe1.ins, sync=True,
                       reason="bulk weights after data loads")

    mm = nc.tensor.matmul
    act = nc.scalar.activation
    tt_ = nc.vector.tensor_tensor
    tcp = nc.vector.tensor_copy
    ptt = nc.gpsimd.tensor_tensor
    pts = nc.gpsimd.tensor_scalar

    # dummy Silu primes the act table with the set that holds tanh+abs+sin
    # +square, so the whole pre-GELU phase runs on a single table load
    nc.vector.memset(dum_sb[:], 0.0)
    a_dum = act(dum_sb[:], dum_sb[:], AF.Silu, bias=zero1_sb[:])

    # ---------------- own phases -> Q ----------------
    ph_ps = pa.tile([P, T], f32, tag="pa")
    for kt in range(2):
        mm(ph_ps[:], _r(kw_sb[:, P * kt: P * kt + P]),
           _r(xT_sb[:, 512 * kt: 512 * kt + 512]),
           start=(kt == 0), stop=(kt == 1))
    a_tanh = act(tv_sb[P:2 * P, :], ph_ps[:], AF.Tanh, bias=kbc_sb[:])
    add_dep_helper(a_tanh.ins, a_dum.ins, sync=True,
                   reason="act table: silu table covers tanh/abs/sin")

    # ---------------- prefix phases -> Kpre ----------------
    pp1 = pa.tile([P, 512], f32, tag="pa")
    pp2 = pa.tile([P, 256], f32, tag="pa")
    for kt in range(2):
        mm(pp1[:], kwb_sb[:, P * kt: P * kt + P],
           xpeT_sb[:, 768 * kt: 768 * kt + 512],
           start=(kt == 0), stop=(kt == 1))
    for kt in range(2):
        mm(pp2[:], kwb_sb[:, P * kt: P * kt + P],
           xpeT_sb[:, 768 * kt + 512: 768 * kt + 768],
           start=(kt == 0), stop=(kt == 1))
    # own-token chain first (it gates scores/retrieved); prefix chain has
    # ~4us more slack before the cross term is needed
    act(tv_sb[0:P, :], tv_sb[P:2 * P, :], AF.Abs, bias=zerop_sb[:])
    # cos(pi*t) = sin(pi/2 - pi*|t|); sin table argument stays in range
    act(qb_sb[0:P, :], tv_sb[0:P, :], AF.Sin,
        bias=halfpi_sb[:], scale=-PI)
    a_oq = act(qb_sb[P:2 * P, :], tv_sb[P:2 * P, :], AF.Sin,
               bias=zerop_sb[:], scale=PI)
    a_pt1 = act(tvp_sb[P:2 * P, 0:512], pp1[:], AF.Tanh, bias=kbc_sb[:])
    add_dep_helper(a_pt1.ins, a_oq.ins, sync=True,
                   reason="own qb before prefix acts on the scalar queue")
    act(tvp_sb[P:2 * P, 512:768], pp2[:], AF.Tanh, bias=kbc_sb[:])
    act(tvp_sb[0:P, :], tvp_sb[P:2 * P, :], AF.Abs, bias=zerop_sb[:])
    act(kpre_sb[0:P, :], tvp_sb[0:P, :], AF.Sin, bias=halfpi_sb[:], scale=-PI)
    act(kpre_sb[P:2 * P, :], tvp_sb[P:2 * P, :], AF.Sin, bias=zerop_sb[:],
        scale=PI)

    # ---------------- V at odd own tokens ----------------
    for blk in range(2):
        vp = pa.tile([128, D], f32, tag="pa")
        for kt in range(2):
            mm(vp[:], _r(xT_sb[:, 512 * kt + 256 * blk + 1:
                               512 * kt + 256 * blk + 256: 2]),
               _r(vw_sb[:, 256 * kt: 256 * kt + 256]),
               start=(kt == 0), stop=(kt == 1))
        nc.scalar.copy(vodd_sb[:, 256 * blk: 256 * blk + 256], vp[:])

    # ---------------- scores (odd tk only) + causal mask ----------------
    sc0 = pb.tile([128, 512], f32, tag="pb")
    mm(sc0[:], _r(qb_sb[:, 0:255:2]), _r(qb_sb[:]), start=True, stop=True)
    tt_(ss0_sb[:, 0:256], sc0[:, 0:256], m0_sb[:], OP.mult)
    nc.scalar.copy(ss0_sb[:, 256:512], sc0[:, 256:512])
    sc1 = pa.tile([128, 256], f32, tag="pa")
    mm(sc1[:], _r(qb_sb[:, 256:511:2]), _r(qb_sb[:, 256:512]),
       start=True, stop=True)
    tt_(ss1_sb[:], sc1[:], m0_sb[:], OP.mult)

    # ---------------- Kpre^T, G, S ----------------
    kprt = pt.tile([128, 6 * 64], bf16, tag="ptr")
    for j in range(6):
        nc.tensor.transpose(kprt[:, 64 * j: 64 * j + 64],
                            kpre_sb[:, 128 * j: 128 * j + 128],
                            eyeb_sb[:])
    tcp(kpreT_sb[:], kprt[:])
    for dh in range(2):
        gp = pa.tile([128, 64], f32, tag="pa2", bufs=1)
        for j in range(6):
            mm(gp[:], xpo_sb[:, 256 * j + 128 * dh: 256 * j + 128 * dh + 128],
               kpreT_sb[:, 64 * j: 64 * j + 64],
               start=(j == 0), stop=(j == 5))
        tcp(gT_sb[:, 64 * dh: 64 * dh + 64], gp[:])
    s_ps = pa.tile([2 * P, D], f32, tag="pa")
    for kt in range(2):
        mm(s_ps[:], _r(gT_sb[:, 64 * kt: 64 * kt + 64]),
           _r(vw_sb[:, 256 * kt: 256 * kt + 256]),
           start=(kt == 0), stop=(kt == 1))
    tcp(s_sb[:], s_ps[:])

    # ---------------- retrieved^T = V^T s + S^T Q ----------------
    retr = []
    retr_cross = []
    for dh in range(2):
        rp = pb.tile([128, 512], f32, tag="pb")
        mm(rp[:, 0:256], _r(vodd_sb[:, 128 * dh: 128 * dh + 128]),
           _r(ss0_sb[:, 0:256]), start=True, stop=True)
        mm(rp[:, 256:512], _r(vodd_sb[:, 128 * dh: 128 * dh + 128]),
           _r(ss0_sb[:, 256:512]), start=True, stop=False)
        mm(rp[:, 256:512], _r(vodd_sb[:, 256 + 128 * dh: 256 + 128 * dh + 128]),
           _r(ss1_sb[:]), start=False, stop=True)
        cp = pa.tile([128, 512], f32, tag="pa")
        mm(cp[:], _r(s_sb[:, 128 * dh: 128 * dh + 128]), _r(qb_sb[:]),
           start=True, stop=True)
        nc.scalar.copy(cross_sb[:, 512 * dh: 512 * dh + 512], cp[:])
        retr.append(rp)
        retr_cross.append(cross_sb[:, 512 * dh: 512 * dh + 512])

    # ---------------- LN1 (folded) ----------------
    def ln_stats(src_ps, src_sb, sqt, stp, msqp, var_sb, m2v_sb,
                 mean_sb, badd, src_add=None):
        # copies src psum -> src_sb (bf16, + optional per-partition bias /
        # cross add), squares, stats; produces the mean row (bf16) and the
        # var row (+eps). With badd, the square reads the psum directly
        # (Square(x + b)) so it does not wait for the copy.
        for dh in range(2):
            sl = slice(512 * dh, 512 * dh + 512)
            if src_add is not None:
                tt_(src_sb[:, sl], src_ps[dh][:], src_add[dh], OP.add)
                act(sqt[:, sl], src_sb[:, sl], AF.Square, bias=zero128_sb[:])
            elif badd is None:
                tcp(src_sb[:, sl], src_ps[dh][:])
                act(sqt[:, sl], src_sb[:, sl], AF.Square, bias=zero128_sb[:])
            else:
                nc.vector.tensor_scalar(
                    out=src_sb[:, sl], in0=src_ps[dh][:],
                    scalar1=badd[:, dh: dh + 1], scalar2=None, op0=OP.add)
                act(sqt[:, sl], src_ps[dh][:], AF.Square,
                    bias=badd[:, dh: dh + 1])
        for kt in range(2):
            mm(stp[0:1, :], invdb_sb[:, 0:1],
               src_sb[:, 512 * kt: 512 * kt + 512],
               start=(kt == 0), stop=(kt == 1))
        for kt in range(2):
            mm(msqp[0:1, :], invdb_sb[:, 0:1],
               sqt[:, 512 * kt: 512 * kt + 512],
               start=(kt == 0), stop=(kt == 1))
        tcp(mean_sb[:], stp[0:1, :])
        act(m2v_sb[:], stp[0:1, :], AF.Square, bias=zero1_sb[:])
        nc.vector.scalar_tensor_tensor(
            out=var_sb[:], in0=msqp[0:1, :], scalar=1e-5, in1=m2v_sb[:],
            op0=OP.add, op1=OP.subtract)

    def ln_rstd(var_sb, stdc, rstdc, nwt):
        # var row -> columns [128,4]; rstd = rsqrt(var) via magic-seed
        # Newton (2 iters). Seed bit-tricks on DVE; float iterations on the
        # pool engine as pure TensorTensor ops (the only elementwise form
        # gpsimd ucode supports).
        vc = pt.tile([128, 4], f32, tag="ptr")
        for j in range(4):
            nc.tensor.transpose(vc[:, j: j + 1],
                                var_sb[0:1, 128 * j: 128 * j + 128],
                                eyef_sb[0:1, 0:1])
        i32 = mybir.dt.int32
        tcp(stdc[:], vc[:])
        nc.vector.tensor_scalar(out=rstdc[:].bitcast(i32),
                                in0=stdc[:].bitcast(i32), scalar1=1,
                                scalar2=None, op0=OP.logical_shift_right)
        nc.vector.tensor_scalar(out=rstdc[:].bitcast(i32),
                                in0=rstdc[:].bitcast(i32), scalar1=-1,
                                scalar2=0x5F3759DF, op0=OP.mult, op1=OP.add)
        for _ in range(1):
            tt_(nwt[:], rstdc[:], rstdc[:], OP.mult)
            nc.vector.scalar_tensor_tensor(
                out=nwt[:], in0=nwt[:], scalar=-0.5, in1=stdc[:],
                op0=OP.mult, op1=OP.mult)
            nc.vector.tensor_scalar(out=nwt[:], in0=nwt[:], scalar1=1.5,
                                    scalar2=None, op0=OP.add)
            tt_(rstdc[:], rstdc[:], nwt[:], OP.mult)

    st1 = pa.tile([1, T], f32, tag="pa")
    ms1 = pa.tile([1, T], f32, tag="pa2", bufs=1)
    ln_stats(retr, r_sb, sq_sb, st1, ms1, var1_sb, m2_sb,
             mean1_sb, None, src_add=retr_cross)

    # W1 on raw r (rstd folded after the matmul):
    #   (W1g^T r - mean*c1) * rstd == W1g^T(LN1(r))
    # so the big matmuls run concurrently with the rstd row computation.
    def w1_block(m):
        hp = pb.tile([128, 512], f32, tag="pb")
        for kt in range(2):
            mm(hp[:], w1g_sb[:, 512 * kt + 128 * m: 512 * kt + 128 * m + 128],
               r_sb[:, 512 * kt: 512 * kt + 512],
               start=(kt == 0), stop=False)
        mm(hp[:], c1n_sb[0:1, 128 * m: 128 * m + 128], mean1_sb[:],
           start=False, stop=True)
        return hp

    hps = [w1_block(0)]
    ln_rstd(var1_sb, stdc_sb, rstdc_sb, nwt1_sb)
    hps.append(w1_block(1))
    # rstd1 broadcast row [128, T]
    rr = pa.tile([1, T], f32, tag="pa")
    for j in range(4):
        nc.tensor.transpose(rr[0:1, 128 * j: 128 * j + 128],
                            rstdc_sb[:, j: j + 1], eyef_sb[:])
    tcp(rstd1_sb[:], rr[:])
    rb1 = pb.tile([128, 512], f32, tag="pb")
    mm(rb1[:], _r(ones_sb[:]), _r(rstd1_sb[:]), start=True, stop=True)
    nc.scalar.copy(rb1s_sb[:], rb1[:])
    for m in range(4):
        if m >= len(hps):
            hps.append(w1_block(m))
        hp = hps[m]
        tt_(hi_sb[:, 512 * m: 512 * m + 512], hp[:], rb1s_sb[:], OP.mult)
        act(h_sb[:, 512 * m: 512 * m + 512],
            hi_sb[:, 512 * m: 512 * m + 512], AF.Gelu,
            bias=cb1_sb[:, m: m + 1])

    # ---------------- W2 -> refined ----------------
    ref = []
    for dh in range(2):
        fp = pb.tile([128, 512], f32, tag="pb")
        for kt in range(4):
            mm(fp[:], w2_sb[:, 256 * kt + 128 * dh: 256 * kt + 128 * dh + 128],
               h_sb[:, 512 * kt: 512 * kt + 512],
               start=(kt == 0), stop=(kt == 3))
        ref.append(fp)

    # ---------------- LN2 (folded) ----------------
    st2 = pa.tile([1, T], f32, tag="pa")
    ms2 = pa.tile([1, T], f32, tag="pa2", bufs=1)
    ln_stats(ref, f_sb, sq2_sb, st2, ms2, var2_sb, m2b_sb,
             mean2_sb, b2c_sb)

    # Wo (token-major) + rank-1 mean fix; rstd2 is per-partition in
    # token-major space so the LN2 apply fuses into the residual op.
    def wo_block(tm):
        op = pa.tile([128, D], f32, tag="pa")
        for dh in range(2):
            mm(op[:], f_sb[:, 512 * dh + 128 * tm: 512 * dh + 128 * tm + 128],
               wog_sb[:, 256 * dh: 256 * dh + 256],
               start=(dh == 0), stop=False)
        mm(op[:], mean2_sb[0:1, 128 * tm: 128 * tm + 128], c1on_sb[:],
           start=False, stop=True)
        return op

    ops = [wo_block(0)]
    ln_rstd(var2_sb, stdc2_sb, rstdc2_sb, nwt2_sb)
    for tm in range(4):
        if tm >= len(ops):
            ops.append(wo_block(tm))
        nc.vector.scalar_tensor_tensor(
            out=out_sb[:, 256 * tm: 256 * tm + 256], in0=ops[tm][:],
            scalar=rstdc2_sb[:, tm: tm + 1],
            in1=xc_sb[:, 256 * tm: 256 * tm + 256],
            op0=OP.mult, op1=OP.add)
        dma(outc[128 * tm: 128 * tm + 128, :],
            out_sb[:, 256 * tm: 256 * tm + 256])

    pb.release()
    pa.release()
    pt.release()
    sb.release()


_CACHE = {}


def _get_nc():
    if "nc" not in _CACHE:
        _CACHE["nc"] = _build()
    return _CACHE["nc"]


def _bf(a):
    return np.asarray(a, np.float32).astype(ml_dtypes.bfloat16)


def kernel(**inputs):
    x = np.asarray(inputs["x"], np.float32)
    key_W = np.asarray(inputs["key_W"], np.float32)
    key_b = np.asarray(inputs["key_b"], np.float32)
    val_W = np.asarray(inputs["val_W"], np.float32)
    val_b = np.asarray(inputs["val_b"], np.float32)
    ln1_g = np.asarray(inputs["ln1_g"], np.float32)
    ln1_b = np.asarray(inputs["ln1_b"], np.float32)
    W1 = np.asarray(inputs["W1"], np.float32)
    b1 = np.asarray(inputs["b1"], np.float32)
    W2 = np.asarray(inputs["W2"], np.float32)
    b2 = np.asarray(inputs["b2"], np.float32)
    ln2_g = np.asarray(inputs["ln2_g"], np.float32)
    ln2_b = np.asarray(inputs["ln2_b"], np.float32)
    Wo = np.asarray(inputs["Wo"], np.float32)
    bo = np.asarray(inputs["bo"], np.float32)

    # these are identically zero for this module; the kernel folds them out
    assert np.allclose(val_b, 0.0), "nonzero val_b unsupported"
    assert np.allclose(bo + ln2_b @ Wo, 0.0), "nonzero output bias unsupported"

    w1g = ln1_g[:, None] * W1
    wog = ln2_g[:, None] * Wo
    m0 = (np.arange(1, 256, 2)[:, None] <=
          np.arange(256)[None, :]).astype(np.float32)
    cpack = np.concatenate([m0, np.eye(128, dtype=np.float32)], axis=1)
    cr1 = np.concatenate([-w1g.sum(0), -wog.sum(0)])[None, :]
    cc1 = np.concatenate([
        (b1 + ln1_b @ W1).reshape(128, 4, order="F"),
        b2.reshape(128, 2, order="F")], axis=1).astype(np.float32)
    shared = {
        "kw": key_W, "kwb": _bf(key_W), "kbc": key_b.reshape(P, 1),
        "vw": val_W,
        "w1g": _bf(w1g),
        "w2": _bf(W2),
        "wog": _bf(wog),
        "cpack": cpack, "cr1": _bf(cr1), "cc1": cc1,
    }
    in_maps = []
    for i in range(N_CORES):
        b, c = divmod(i, 4)
        l0 = c * T
        npairs = l0 // 2
        xpe = np.zeros((PRE, D), np.float32)
        xpo = np.zeros((PRE, D), np.float32)
        if npairs:
            xpe[:npairs] = x[b, 0:l0 - 1:2]
            xpo[:npairs] = x[b, 1:l0:2]
        xch = np.ascontiguousarray(x[b, l0:l0 + T])
        in_maps.append({
            "xc": xch,
            "xt": np.ascontiguousarray(xch.T),
            "xpet": _bf(np.ascontiguousarray(xpe.T)),
            "xpo": _bf(xpo), **shared,
        })

    nc = _get_nc()
    res = run_bass_kernel_spmd(nc, in_maps, core_ids=list(range(N_CORES)),
                               **_CACHE.get("run_kwargs", {}))
    _CACHE["last_result"] = res
    out = np.empty((B, L, D), np.float32)
    for i in range(N_CORES):
        b, c = divmod(i, 4)
        out[b, c * T:(c + 1) * T] = res.results[i]["outc"]
    return out



# revision 9
# speedup vs baseline: 1.0440x; 1.0440x over previous
"""Trainium2 Bass kernel for nn_HardcodedKVMemoryBlock (8 NeuronCores).

Sharding: core i handles batch b=i//4, sequence chunk c=i%4 (512 tokens).
The (B,L,P,D) cumsum is restructured as causal linear attention:
    retrieved = tril(Q K^T) @ V + Q @ S_prefix
with Q=[cos,sin] phasors (L x 64), V = values at odd positions, and the
cross-chunk carry S_prefix = (K_even^T @ x_odd) @ val_W computed
redundantly per core from a zero-padded prefix (no collectives).
The 1/sqrt(valid*P) normalization cancels inside LayerNorm1 (scale
invariance); ln gains are folded into W1/Wo and means are applied as
rank-1 PE updates.

v4 notes:
- x and phasors in fp16 (8x tighter mantissa than bf16, same DMA bytes)
- inputs are host-prepped SBUF images (one packet per partition)
- NO DMA descriptors on the scalar queue (descriptor gen is ~650ns each
  and was starving the act-table load); consts packed into one tensor
- 9 warmup matmuls guarantee >3.4us of continuous PE busy so the HAM
  clock gate releases (1.2 -> 2.4 GHz) before the real stream
- prefix phases token-major (cheap narrow acts, kpre born j-major)
- LN1 applied by pre-scaling r with the rstd broadcast, so GELU reads
  the W1 PSUM directly (bias only) - no intermediate hi tile
- LN2 rstd Newton runs on DVE underneath the Wo matmuls; rstd2 is
  applied inside the residual scalar_tensor_tensor
- casts/squares for LN stats split across scalar and DVE queues
"""

import math
import numpy as np
import ml_dtypes

import concourse.bass as bass
import concourse.tile as tile
from concourse import bacc, mybir
from concourse.bass_utils import run_bass_kernel_spmd
from concourse.tile import add_dep_helper

PI = math.pi
B, L, D, P = 2, 2048, 256, 32
T = 512          # own tokens per core
H = 512          # MLP hidden
PRE = 768        # padded prefix pair count (max prefix 1536 tokens / 2)
N_CORES = 8
WARM_N = 9       # PE warmup matmuls (512 cols each)

f32 = mybir.dt.float32
f32r = mybir.dt.float32r
bf16 = mybir.dt.bfloat16
f16 = mybir.dt.float16
i32 = mybir.dt.int32
AF = mybir.ActivationFunctionType
OP = mybir.AluOpType
DUM_AF = AF.Silu    # table-priming dummy (CoreSim debug overrides this)
GELU_AF = AF.Gelu


def _r(ap):
    return ap.bitcast(f32r)


def _build():
    nc = bacc.Bacc("TRN2", target_bir_lowering=False, debug=False,
                   num_devices=N_CORES)

    def din(name, shape, dt):
        return nc.dram_tensor(name, shape, dt, kind="ExternalInput").ap()

    xt = din("xt", [128, 1024], f16)      # x^T image: dh at cols 512*dh
    xc = din("xc", [128, 1024], f16)      # token-major: tt at cols 256*tt
    xpet = din("xpet", [128, 1536], f16)  # prefix even^T: kt at cols 768*kt
    xpo = din("xpo", [128, 1536], f16)    # prefix odd: jb at cols 256*jb
    kwb = din("kwb", [128, 64], f16)      # kt at cols 32*kt
    vw = din("vw", [128, 512], f16)       # kt at cols 256*kt
    w1g = din("w1g", [128, 1024], bf16)   # kt at cols 512*kt
    w2 = din("w2", [128, 1024], bf16)     # h-block kt at cols 256*kt
    wog = din("wog", [128, 512], bf16)    # dh at cols 256*dh
    cpk = din("cpk", [128, 390], f32)     # m0 | eyef | cb1 | b2c
    cr1 = din("cr1", [1, H + D], bf16)    # c1n | c1on
    outc = nc.dram_tensor("outc", [T, D], f32, kind="ExternalOutput").ap()

    with tile.TileContext(nc) as tc:
        _emit(tc, locals())
    nc.compile()
    return nc


def _emit(tc, io):
    nc = tc.nc
    outc = io["outc"]

    sb = tc.alloc_tile_pool(name="sb", bufs=1)
    pt = tc.alloc_tile_pool(name="pt", bufs=1, space="PSUM")    # small
    pa = tc.alloc_tile_pool(name="pa", bufs=2, space="PSUM")    # narrow
    pb = tc.alloc_tile_pool(name="pb", bufs=3, space="PSUM")    # [128,512]

    # ---------------- SBUF tiles ----------------
    wu_sb = sb.tile([128, 640], bf16)         # warmup stationary+moving
    kwb_sb = sb.tile([128, 64], f16)
    vw_sb = sb.tile([128, 512], f16)
    w1g_sb = sb.tile([128, 1024], bf16)
    w2_sb = sb.tile([128, 1024], bf16)
    wog_sb = sb.tile([128, 512], bf16)
    cpk_sb = sb.tile([128, 390], f32)
    cr1_sb = sb.tile([1, H + D], bf16)
    xt_sb = sb.tile([128, 1024], f16)
    xc_sb = sb.tile([128, 1024], f16)
    xpet_sb = sb.tile([128, 1536], f16)
    xpo_sb = sb.tile([128, 1536], f16)

    tv_sb = sb.tile([64, 512], f32)           # rows 0:32 |t|, 32:64 tanh
    qb_sb = sb.tile([64, 512], f16)           # rows 0:32 cos, 32:64 sin
    tvp_sb = sb.tile([128, 384], f32)         # cols 0:192 tanh, 192:384 abs
    kcs_sb = sb.tile([128, 384], f16)         # cols 0:192 cos, 192:384 sin
    vodd_sb = sb.tile([128, 512], f16)        # blk at cols 256*blk
    gT_sb = sb.tile([128, 128], f16)          # dh at cols 64*dh
    s_sb = sb.tile([64, 256], f16)
    ss_sb = sb.tile([128, 768], f16)          # ss0 (512) | ss1 (256)
    r_sb = sb.tile([128, 1024], bf16)         # dh at cols 512*dh
    sq_sb = sb.tile([128, 1024], bf16)
    rs_sb = sb.tile([128, 1024], bf16)        # r * rstd1 (pre-scaled)
    rb1s_sb = sb.tile([128, 512], f32)
    h_sb = sb.tile([128, 2048], bf16)         # m at cols 512*m
    f_sb = sb.tile([128, 1024], bf16)
    sq2_sb = sb.tile([128, 1024], bf16)
    out_sb = sb.tile([128, 1024], f32)        # tt at cols 256*tt

    mean1_sb = sb.tile([1, T], bf16)
    mr1_sb = sb.tile([1, T], bf16)            # mean1 * rstd1
    var1_sb = sb.tile([1, T], f32)
    m2_sb = sb.tile([1, T], f32)
    rstd1_sb = sb.tile([1, T], f32r)
    mean2_sb = sb.tile([1, T], bf16)
    var2_sb = sb.tile([1, T], f32)
    m2b_sb = sb.tile([1, T], f32)
    stdc_sb = sb.tile([128, 4], f32)
    rstdc_sb = sb.tile([128, 4], f32)
    nwt1_sb = sb.tile([128, 4], f32)
    stdc2_sb = sb.tile([128, 4], f32)
    rstdc2_sb = sb.tile([128, 4], f32)
    nwt2_sb = sb.tile([128, 4], f32)

    ones_sb = sb.tile([1, 128], f32r)
    invdb_sb = sb.tile([128, 1], bf16)
    halfpi_sb = sb.tile([128, 1], f32)
    zerop_sb = sb.tile([128, 1], f32)
    zero1_sb = sb.tile([1, 1], f32)
    dum_sb = sb.tile([1, 1], f32)

    m0_sb = cpk_sb[:, 0:256]
    eyef_sb = cpk_sb[:, 256:384]
    cb1_sb = cpk_sb[:, 384:388]
    b2c_sb = cpk_sb[:, 388:390]
    c1n_sb = cr1_sb[:, 0:H]
    c1on_sb = cr1_sb[:, H:H + D]

    mm = nc.tensor.matmul
    act = nc.scalar.activation
    tt_ = nc.vector.tensor_tensor
    tcp = nc.vector.tensor_copy
    ts_ = nc.vector.tensor_scalar
    stt = nc.vector.scalar_tensor_tensor

    # warmup source first so the PE can start immediately
    nc.vector.memset(wu_sb[:], 0.0)
    nc.vector.memset(halfpi_sb[:], PI / 2)
    nc.vector.memset(zerop_sb[:], 0.0)
    nc.vector.memset(zero1_sb[:], 0.0)
    nc.vector.memset(dum_sb[:], 0.0)
    nc.vector.memset(ones_sb[:].bitcast(f32), 1.0)
    nc.vector.memset(invdb_sb[:], 1.0 / D)

    # -------- DMA issues (sync + gpsimd only; scalar runs acts) --------
    dma_s = nc.sync.dma_start
    dma_g = nc.gpsimd.dma_start
    dma_g(kwb_sb[:], io["kwb"])
    d_xt0 = dma_s(xt_sb[:, 0:512], io["xt"][:, 0:512])
    dma_g(vw_sb[:], io["vw"])
    d_xt1 = dma_s(xt_sb[:, 512:1024], io["xt"][:, 512:1024])
    dma_g(xpet_sb[:], io["xpet"])
    dma_s(w1g_sb[:], io["w1g"])
    dma_g(xpo_sb[:], io["xpo"])
    dma_s(cpk_sb[:], io["cpk"])
    dma_g(cr1_sb[:], io["cr1"])
    dma_s(w2_sb[:], io["w2"])
    dma_g(xc_sb[:], io["xc"])
    dma_s(wog_sb[:], io["wog"])

    # ------------ PE warmup (release the HAM clock gate) ------------
    wu_ps = pt.tile([128, 512], f32, tag="pt")
    for i in range(WARM_N):
        mm(wu_ps[:], wu_sb[:, 0:128], wu_sb[:, 128:640],
           start=True, stop=True)

    # prime the silu act table (covers tanh/sin/square) exactly once
    a_dum = act(dum_sb[:], dum_sb[:], DUM_AF, bias=zero1_sb[:])

    # ---------------- own phases -> Q ----------------
    ph_ps = pa.tile([P, T], f32, tag="pa")
    for kt in range(2):
        mm(ph_ps[:], kwb_sb[:, 32 * kt: 32 * kt + 32],
           xt_sb[:, 512 * kt: 512 * kt + 512],
           start=(kt == 0), stop=(kt == 1))
    a_tanh = act(tv_sb[P:2 * P, :], ph_ps[:], AF.Tanh, bias=zerop_sb[0:P, :])
    add_dep_helper(a_tanh.ins, a_dum.ins, sync=True,
                   reason="act table: silu table covers tanh/sin/square")
    ts_(out=tv_sb[0:P, :].bitcast(i32), in0=tv_sb[P:2 * P, :].bitcast(i32),
        scalar1=0x7FFFFFFF, scalar2=None, op0=OP.bitwise_and)
    # cos(pi*t) = sin(pi/2 - pi*|t|); sin table argument stays in range
    act(qb_sb[0:P, :], tv_sb[0:P, :], AF.Sin,
        bias=halfpi_sb[0:P, :], scale=-PI)
    a_oq = act(qb_sb[P:2 * P, :], tv_sb[P:2 * P, :], AF.Sin,
               bias=zerop_sb[0:P, :], scale=PI)

    # ---------------- V at odd own tokens ----------------
    for blk in range(2):
        vp = pa.tile([128, D], f32, tag="pa")
        for kt in range(2):
            mm(vp[:], xt_sb[:, 512 * kt + 256 * blk + 1:
                            512 * kt + 256 * blk + 256: 2],
               vw_sb[:, 256 * kt: 256 * kt + 256],
               start=(kt == 0), stop=(kt == 1))
        tcp(vodd_sb[:, 256 * blk: 256 * blk + 256], vp[:])

    # ---------------- prefix phases (token-major) -> kpre --------------
    pp_ps = pa.tile([128, 192], f32, tag="pa")
    for j in range(6):
        for kt in range(2):
            mm(pp_ps[:, 32 * j: 32 * j + 32],
               xpet_sb[:, 768 * kt + 128 * j: 768 * kt + 128 * j + 128],
               kwb_sb[:, 32 * kt: 32 * kt + 32],
               start=(kt == 0), stop=(kt == 1))
    a_pt = act(tvp_sb[:, 0:192], pp_ps[:], AF.Tanh, bias=zerop_sb[:])
    add_dep_helper(a_pt.ins, a_oq.ins, sync=True,
                   reason="own qb before prefix acts on the scalar queue")
    ts_(out=tvp_sb[:, 192:384].bitcast(i32), in0=tvp_sb[:, 0:192].bitcast(i32),
        scalar1=0x7FFFFFFF, scalar2=None, op0=OP.bitwise_and)
    act(kcs_sb[:, 0:192], tvp_sb[:, 192:384], AF.Sin,
        bias=halfpi_sb[:], scale=-PI)
    act(kcs_sb[:, 192:384], tvp_sb[:, 0:192], AF.Sin,
        bias=zerop_sb[:], scale=PI)

    # ---------------- scores (odd tk only) + causal mask ----------------
    sc0 = pb.tile([128, 512], f32, tag="pb")
    mm(sc0[:], qb_sb[:, 0:255:2], qb_sb[:], start=True, stop=True)
    tt_(ss_sb[:, 0:256], sc0[:, 0:256], m0_sb[:], OP.mult)
    tcp(ss_sb[:, 256:512], sc0[:, 256:512])
    sc1 = pa.tile([128, 256], f32, tag="pa")
    mm(sc1[:], qb_sb[:, 256:511:2], qb_sb[:, 256:512], start=True, stop=True)
    tt_(ss_sb[:, 512:768], sc1[:], m0_sb[:], OP.mult)

    # ---------------- prefix G and S ----------------
    # cos/sin accumulation groups live in different PSUM banks (pt vs pa
    # pools) so they can interleave while sharing the xpo stationary.
    for dh in range(2):
        gpc = pt.tile([128, 32], f32, tag="pt")
        gps = pa.tile([128, 32], f32, tag="pa")
        for j in range(6):
            mm(gpc[:],
               xpo_sb[:, 256 * j + 128 * dh: 256 * j + 128 * dh + 128],
               kcs_sb[:, 32 * j: 32 * j + 32],
               start=(j == 0), stop=(j == 5))
            mm(gps[:],
               xpo_sb[:, 256 * j + 128 * dh: 256 * j + 128 * dh + 128],
               kcs_sb[:, 192 + 32 * j: 192 + 32 * j + 32],
               start=(j == 0), stop=(j == 5))
        tcp(gT_sb[:, 64 * dh: 64 * dh + 32], gpc[:])
        tcp(gT_sb[:, 64 * dh + 32: 64 * dh + 64], gps[:])
    s_ps = pa.tile([2 * P, D], f32, tag="pa")
    for kt in range(2):
        mm(s_ps[:], gT_sb[:, 64 * kt: 64 * kt + 64],
           vw_sb[:, 256 * kt: 256 * kt + 256],
           start=(kt == 0), stop=(kt == 1))
    tcp(s_sb[:], s_ps[:])

    # ------------- retrieved^T = tril(V^T ss) + S^T Q -------------
    retr = []
    for dh in range(2):
        rp = pb.tile([128, 512], f32, tag="pb")
        # one PSUM bank allows a single open accumulation group: close the
        # first-half group before opening the second-half one
        mm(rp[:, 0:256], vodd_sb[:, 128 * dh: 128 * dh + 128],
           ss_sb[:, 0:256], start=True, stop=False)
        mm(rp[:, 0:256], s_sb[:, 128 * dh: 128 * dh + 128],
           qb_sb[:, 0:256], start=False, stop=True)
        mm(rp[:, 256:512], vodd_sb[:, 128 * dh: 128 * dh + 128],
           ss_sb[:, 256:512], start=True, stop=False)
        mm(rp[:, 256:512], vodd_sb[:, 256 + 128 * dh: 256 + 128 * dh + 128],
           ss_sb[:, 512:768], start=False, stop=False)
        mm(rp[:, 256:512], s_sb[:, 128 * dh: 128 * dh + 128],
           qb_sb[:, 256:512], start=False, stop=True)
        retr.append(rp)

    # ------- LN1 stats: casts/squares split over DVE and scalar -------
    tcp(r_sb[:, 0:512], retr[0][:])                       # DVE cast
    a_c1 = act(r_sb[:, 512:1024], retr[1][:], AF.Identity,
               bias=zerop_sb[:])                          # scalar copy
    act(sq_sb[:, 0:512], retr[0][:], AF.Square, bias=zerop_sb[:])
    tt_(sq_sb[:, 512:1024], r_sb[:, 512:1024], r_sb[:, 512:1024], OP.mult)
    st1 = pa.tile([1, T], f32, tag="pa")
    ms1 = pa.tile([1, T], f32, tag="pa2", bufs=1)
    for kt in range(2):
        mm(st1[0:1, :], invdb_sb[:, 0:1], r_sb[:, 512 * kt: 512 * kt + 512],
           start=(kt == 0), stop=(kt == 1))
    for kt in range(2):
        mm(ms1[0:1, :], invdb_sb[:, 0:1], sq_sb[:, 512 * kt: 512 * kt + 512],
           start=(kt == 0), stop=(kt == 1))
    tcp(mean1_sb[:], st1[0:1, :])
    tt_(m2_sb[:], mean1_sb[:], mean1_sb[:], OP.mult)
    stt(out=var1_sb[:], in0=ms1[0:1, :], scalar=1e-5, in1=m2_sb[:],
        op0=OP.add, op1=OP.subtract)

    def ln_rstd(var_sb, stdc, rstdc, nwt):
        # var row -> columns [128,4]; rstd = rsqrt(var) via magic-seed
        # Newton (1 iter), float work on DVE.
        vc = pt.tile([128, 4], f32, tag="pt")
        for j in range(4):
            nc.tensor.transpose(vc[:, j: j + 1],
                                var_sb[0:1, 128 * j: 128 * j + 128],
                                eyef_sb[0:1, 0:1])
        tcp(stdc[:], vc[:])
        ts_(out=rstdc[:].bitcast(i32), in0=stdc[:].bitcast(i32), scalar1=1,
            scalar2=None, op0=OP.logical_shift_right)
        ts_(out=rstdc[:].bitcast(i32), in0=rstdc[:].bitcast(i32), scalar1=-1,
            scalar2=0x5F3759DF, op0=OP.mult, op1=OP.add)
        tt_(nwt[:], rstdc[:], rstdc[:], OP.mult)
        stt(out=nwt[:], in0=nwt[:], scalar=-0.5, in1=stdc[:],
            op0=OP.mult, op1=OP.mult)
        ts_(out=nwt[:], in0=nwt[:], scalar1=1.5, scalar2=None, op0=OP.add)
        tt_(rstdc[:], rstdc[:], nwt[:], OP.mult)

    # rstd1 row + broadcast, then pre-scale r:  rs = r * rstd1
    ln_rstd(var1_sb, stdc_sb, rstdc_sb, nwt1_sb)
    rr = pt.tile([1, T], f32, tag="pt")
    for j in range(4):
        nc.tensor.transpose(rr[0:1, 128 * j: 128 * j + 128],
                            rstdc_sb[:, j: j + 1], eyef_sb[:])
    tcp(rstd1_sb[:], rr[:])
    rb1 = pb.tile([128, 512], f32, tag="pb")
    mm(rb1[:], _r(ones_sb[:]), _r(rstd1_sb[:]), start=True, stop=True)
    tcp(rb1s_sb[:], rb1[:])
    tt_(mr1_sb[:], mean1_sb[:], rstd1_sb[:].bitcast(f32), OP.mult)
    tt_(rs_sb[:, 0:512], retr[0][:], rb1s_sb[:], OP.mult)
    tt_(rs_sb[:, 512:1024], retr[1][:], rb1s_sb[:], OP.mult)

    # --------------- W1 on pre-scaled rs; GELU from PSUM ---------------
    #   W1g^T rs + c1n (x) (mean1*rstd1) == W1g^T(LN1(r))
    for m in range(4):
        hp = pb.tile([128, 512], f32, tag="pb")
        for kt in range(2):
            mm(hp[:], w1g_sb[:, 512 * kt + 128 * m: 512 * kt + 128 * m + 128],
               rs_sb[:, 512 * kt: 512 * kt + 512],
               start=(kt == 0), stop=False)
        mm(hp[:], c1n_sb[0:1, 128 * m: 128 * m + 128], mr1_sb[:],
           start=False, stop=True)
        act(h_sb[:, 512 * m: 512 * m + 512], hp[:], GELU_AF,
            bias=cb1_sb[:, m: m + 1])

    # ---------------- W2 -> refined ----------------
    ref = []
    for dh in range(2):
        fp = pb.tile([128, 512], f32, tag="pb")
        for kt in range(4):
            mm(fp[:], w2_sb[:, 256 * kt + 128 * dh: 256 * kt + 128 * dh + 128],
               h_sb[:, 512 * kt: 512 * kt + 512],
               start=(kt == 0), stop=(kt == 3))
        ref.append(fp)

    # ------- LN2 stats (+b2 bias), split over DVE and scalar -------
    ts_(out=f_sb[:, 0:512], in0=ref[0][:], scalar1=b2c_sb[:, 0:1],
        scalar2=None, op0=OP.add)                          # DVE
    act(f_sb[:, 512:1024], ref[1][:], AF.Identity, bias=b2c_sb[:, 1:2])
    act(sq2_sb[:, 0:512], ref[0][:], AF.Square, bias=b2c_sb[:, 0:1])
    tt_(sq2_sb[:, 512:1024], f_sb[:, 512:1024], f_sb[:, 512:1024], OP.mult)
    st2 = pa.tile([1, T], f32, tag="pa")
    ms2 = pa.tile([1, T], f32, tag="pa2", bufs=1)
    for kt in range(2):
        mm(st2[0:1, :], invdb_sb[:, 0:1], f_sb[:, 512 * kt: 512 * kt + 512],
           start=(kt == 0), stop=(kt == 1))
    for kt in range(2):
        mm(ms2[0:1, :], invdb_sb[:, 0:1], sq2_sb[:, 512 * kt: 512 * kt + 512],
           start=(kt == 0), stop=(kt == 1))
    tcp(mean2_sb[:], st2[0:1, :])
    tt_(m2b_sb[:], mean2_sb[:], mean2_sb[:], OP.mult)
    stt(out=var2_sb[:], in0=ms2[0:1, :], scalar=1e-5, in1=m2b_sb[:],
        op0=OP.add, op1=OP.subtract)

    # Wo (token-major) + rank-1 mean fix; Newton for rstd2 runs on DVE
    # underneath these matmuls; rstd2 applies inside the residual op.
    ops = []
    for tm in range(4):
        op = pa.tile([128, D], f32, tag="pa")
        for dh in range(2):
            mm(op[:], f_sb[:, 512 * dh + 128 * tm: 512 * dh + 128 * tm + 128],
               wog_sb[:, 256 * dh: 256 * dh + 256],
               start=(dh == 0), stop=False)
        mm(op[:], mean2_sb[0:1, 128 * tm: 128 * tm + 128], c1on_sb[:],
           start=False, stop=True)
        ops.append(op)
        if tm == 0:
            ln_rstd(var2_sb, stdc2_sb, rstdc2_sb, nwt2_sb)
        if tm >= 1:
            t0 = tm - 1
            stt(out=out_sb[:, 256 * t0: 256 * t0 + 256], in0=ops[t0][:],
                scalar=rstdc2_sb[:, t0: t0 + 1],
                in1=xc_sb[:, 256 * t0: 256 * t0 + 256],
                op0=OP.mult, op1=OP.add)
            dq = dma_s if t0 % 2 == 0 else dma_g
            dq(outc[128 * t0: 128 * t0 + 128, :],
               out_sb[:, 256 * t0: 256 * t0 + 256])
    stt(out=out_sb[:, 768:1024], in0=ops[3][:],
        scalar=rstdc2_sb[:, 3:4], in1=xc_sb[:, 768:1024],
        op0=OP.mult, op1=OP.add)
    dma_g(outc[384:512, :], out_sb[:, 768:1024])

    pb.release()
    pa.release()
    pt.release()
    sb.release()


_CACHE = {}


def _get_nc():
    if "nc" not in _CACHE:
        _CACHE["nc"] = _build()
    return _CACHE["nc"]


def _bf(a):
    return np.asarray(a, np.float32).astype(ml_dtypes.bfloat16)


def _f16(a):
    return np.asarray(a, np.float32).astype(np.float16)


def _img(a, nblk, w):
    # [nblk*128, w] -> [128, nblk*w] SBUF image (block b at cols w*b)
    a = np.asarray(a, np.float32)
    return np.ascontiguousarray(
        a.reshape(nblk, 128, w).transpose(1, 0, 2).reshape(128, nblk * w))


def kernel(**inputs):
    x = np.asarray(inputs["x"], np.float32)
    key_W = np.asarray(inputs["key_W"], np.float32)
    key_b = np.asarray(inputs["key_b"], np.float32)
    val_W = np.asarray(inputs["val_W"], np.float32)
    val_b = np.asarray(inputs["val_b"], np.float32)
    ln1_g = np.asarray(inputs["ln1_g"], np.float32)
    ln1_b = np.asarray(inputs["ln1_b"], np.float32)
    W1 = np.asarray(inputs["W1"], np.float32)
    b1 = np.asarray(inputs["b1"], np.float32)
    W2 = np.asarray(inputs["W2"], np.float32)
    b2 = np.asarray(inputs["b2"], np.float32)
    ln2_g = np.asarray(inputs["ln2_g"], np.float32)
    ln2_b = np.asarray(inputs["ln2_b"], np.float32)
    Wo = np.asarray(inputs["Wo"], np.float32)
    bo = np.asarray(inputs["bo"], np.float32)

    # these are identically zero for this module; the kernel folds them out
    assert np.allclose(val_b, 0.0), "nonzero val_b unsupported"
    assert np.allclose(key_b, 0.0), "nonzero key_b unsupported"
    assert np.allclose(bo + ln2_b @ Wo, 0.0), "nonzero output bias unsupported"

    w1g = ln1_g[:, None] * W1
    wog = ln2_g[:, None] * Wo
    m0 = (np.arange(1, 256, 2)[:, None] <=
          np.arange(256)[None, :]).astype(np.float32)
    cpk = np.concatenate([
        m0, np.eye(128, dtype=np.float32),
        (b1 + ln1_b @ W1).reshape(128, 4, order="F"),
        b2.reshape(128, 2, order="F")], axis=1).astype(np.float32)
    cr1 = np.concatenate([-w1g.sum(0), -wog.sum(0)])[None, :]
    shared = {
        "kwb": _f16(_img(key_W, 2, P)),
        "vw": _f16(_img(val_W, 2, D)),
        "w1g": _bf(_img(w1g, 2, H)),
        "w2": _bf(_img(W2, 4, D)),
        "wog": _bf(_img(wog, 2, D)),
        "cpk": np.ascontiguousarray(cpk), "cr1": _bf(cr1),
    }
    in_maps = []
    for i in range(N_CORES):
        b, c = divmod(i, 4)
        l0 = c * T
        npairs = l0 // 2
        xpe = np.zeros((PRE, D), np.float32)
        xpo = np.zeros((PRE, D), np.float32)
        if npairs:
            xpe[:npairs] = x[b, 0:l0 - 1:2]
            xpo[:npairs] = x[b, 1:l0:2]
        xch = np.ascontiguousarray(x[b, l0:l0 + T])
        in_maps.append({
            "xt": _f16(_img(xch.T, 2, T)),
            "xc": _f16(_img(xch, 4, D)),
            "xpet": _f16(_img(np.ascontiguousarray(xpe.T), 2, PRE)),
            "xpo": _f16(_img(xpo, 6, D)),
            **shared,
        })

    nc = _get_nc()
    res = run_bass_kernel_spmd(nc, in_maps, core_ids=list(range(N_CORES)),
                               **_CACHE.get("run_kwargs", {}))
    _CACHE["last_result"] = res
    out = np.empty((B, L, D), np.float32)
    for i in range(N_CORES):
        b, c = divmod(i, 4)
        out[b, c * T:(c + 1) * T] = res.results[i]["outc"]
    return out


# revision 12
# speedup vs baseline: 1.0831x; 1.0375x over previous
"""Trainium2 Bass kernel for nn_HardcodedKVMemoryBlock (8 NeuronCores).

Sharding: core i handles batch b=i//4, sequence chunk c=i%4 (512 tokens).
The (B,L,P,D) cumsum is restructured as causal linear attention:
    retrieved = tril(Q K^T) @ V + Q @ S_prefix
with Q=[cos,sin] phasors (L x 64), V = values at odd positions, and the
cross-chunk carry S_prefix = (K_even^T @ x_odd) @ val_W computed
redundantly per core from a zero-padded prefix (no collectives).
The 1/sqrt(valid*P) normalization cancels inside LayerNorm1 (scale
invariance); ln gains are folded into W1/Wo and means are applied as
rank-1 PE updates.

v4 notes:
- x and phasors in fp16 (8x tighter mantissa than bf16, same DMA bytes)
- inputs are host-prepped SBUF images (one packet per partition)
- NO DMA descriptors on the scalar queue (descriptor gen is ~650ns each
  and was starving the act-table load); consts packed into one tensor
- 9 warmup matmuls guarantee >3.4us of continuous PE busy so the HAM
  clock gate releases (1.2 -> 2.4 GHz) before the real stream
- prefix phases token-major (cheap narrow acts, kpre born j-major)
- LN1 applied by pre-scaling r with the rstd broadcast, so GELU reads
  the W1 PSUM directly (bias only) - no intermediate hi tile
- LN2 rstd Newton runs on DVE underneath the Wo matmuls; rstd2 is
  applied inside the residual scalar_tensor_tensor
- casts/squares for LN stats split across scalar and DVE queues
"""

import math
import numpy as np
import ml_dtypes

import concourse.bass as bass
import concourse.tile as tile
from concourse import bacc, mybir
from concourse.bass_utils import run_bass_kernel_spmd
from concourse.tile import add_dep_helper

PI = math.pi
B, L, D, P = 2, 2048, 256, 32
T = 512          # own tokens per core
H = 512          # MLP hidden
PRE = 768        # padded prefix pair count (max prefix 1536 tokens / 2)
N_CORES = 8

f32 = mybir.dt.float32
f32r = mybir.dt.float32r
bf16 = mybir.dt.bfloat16
f16 = mybir.dt.float16
i32 = mybir.dt.int32
AF = mybir.ActivationFunctionType
OP = mybir.AluOpType
DUM_AF = AF.Silu    # table-priming dummy (CoreSim debug overrides this)
GELU_AF = AF.Gelu


def _r(ap):
    return ap.bitcast(f32r)


def _build():
    nc = bacc.Bacc("TRN2", target_bir_lowering=False, debug=False,
                   num_devices=N_CORES)

    def din(name, shape, dt):
        return nc.dram_tensor(name, shape, dt, kind="ExternalInput").ap()

    xt = din("xt", [128, 1024], f16)      # x^T image: dh at cols 512*dh
    xc = din("xc", [128, 1024], f16)      # token-major: tt at cols 256*tt
    xpetl = din("xpetl", [128, 768], f16)   # prefix even^T cols 0:384 per kt
    xpeth = din("xpeth", [128, 768], f16)   # prefix even^T cols 384:768 per kt
    xpol = din("xpol", [128, 768], f16)     # prefix odd jb 0:3
    xpoh = din("xpoh", [128, 768], f16)     # prefix odd jb 3:6
    kwb = din("kwb", [128, 64], f16)      # kt at cols 32*kt
    vw = din("vw", [128, 512], f16)       # kt at cols 256*kt
    w1g = din("w1g", [128, 1024], bf16)   # kt at cols 512*kt
    w2 = din("w2", [128, 1024], bf16)     # h-block kt at cols 256*kt
    wog = din("wog", [128, 512], bf16)    # dh at cols 256*dh
    cpk = din("cpk", [128, 390], f32)     # m0 | eyef | cb1 | b2c
    cr1 = din("cr1", [1, H + D], bf16)    # c1n | c1on
    outc = nc.dram_tensor("outc", [T, D], f32, kind="ExternalOutput").ap()

    with tile.TileContext(nc) as tc:
        _emit(tc, locals())
    nc.compile()
    return nc


def _emit(tc, io):
    nc = tc.nc
    outc = io["outc"]

    sb = tc.alloc_tile_pool(name="sb", bufs=1)
    pt = tc.alloc_tile_pool(name="pt", bufs=1, space="PSUM")    # small
    pa = tc.alloc_tile_pool(name="pa", bufs=2, space="PSUM")    # narrow
    pb = tc.alloc_tile_pool(name="pb", bufs=3, space="PSUM")    # [128,512]

    # ---------------- SBUF tiles ----------------
    wu_sb = sb.tile([128, 640], bf16)         # warmup stationary+moving
    kwb_sb = sb.tile([128, 64], f16)
    vw_sb = sb.tile([128, 512], f16)
    w1g_sb = sb.tile([128, 1024], bf16)
    w2_sb = sb.tile([128, 1024], bf16)
    wog_sb = sb.tile([128, 512], bf16)
    cpk_sb = sb.tile([128, 390], f32)
    cr1_sb = sb.tile([1, H + D], bf16)
    xt_sb = sb.tile([128, 1024], f16)
    xc_sb = sb.tile([128, 1024], f16)
    xpet_sb = sb.tile([128, 1536], f16)
    xpo_sb = sb.tile([128, 1536], f16)

    tv_sb = sb.tile([64, 512], f32)           # rows 0:32 |t|, 32:64 tanh
    qb_sb = sb.tile([64, 512], f16)           # rows 0:32 cos, 32:64 sin
    tvp_sb = sb.tile([128, 384], f32)         # cols 0:192 tanh, 192:384 abs
    kcs_sb = sb.tile([128, 384], f16)         # cols 0:192 cos, 192:384 sin
    vodd_sb = sb.tile([128, 512], f16)        # blk at cols 256*blk
    gT_sb = sb.tile([128, 128], f16)          # dh at cols 64*dh
    s_sb = sb.tile([64, 256], f16)
    ss_sb = sb.tile([128, 768], f16)          # ss0 (512) | ss1 (256)
    r_sb = sb.tile([128, 1024], bf16)         # dh at cols 512*dh
    sq_sb = sb.tile([128, 1024], bf16)
    rs_sb = sb.tile([128, 2048], f16)         # hi staging (hp * rstd1)
    rb1s_sb = sb.tile([128, 512], f32)
    h_sb = sb.tile([128, 2048], bf16)         # m at cols 512*m
    f_sb = sb.tile([128, 1024], bf16)
    sq2_sb = sb.tile([128, 1024], bf16)
    out_sb = sb.tile([128, 1024], f32)        # tt at cols 256*tt

    mean1_sb = sb.tile([1, T], bf16)
    var1_sb = sb.tile([1, T], f32)
    m2_sb = sb.tile([1, T], f32)
    rstd1_sb = sb.tile([1, T], f32r)
    mean2_sb = sb.tile([1, T], bf16)
    var2_sb = sb.tile([1, T], f32)
    m2b_sb = sb.tile([1, T], f32)
    stdc_sb = sb.tile([128, 4], f32)
    rstdc_sb = sb.tile([128, 4], f32)
    nwt1_sb = sb.tile([128, 4], f32)
    stdc2_sb = sb.tile([128, 4], f32)
    rstdc2_sb = sb.tile([128, 4], f32)
    nwt2_sb = sb.tile([128, 4], f32)

    ones_sb = sb.tile([1, 128], f32r)
    invdb_sb = sb.tile([128, 1], bf16)
    halfpi_sb = sb.tile([128, 1], f32)
    zerop_sb = sb.tile([128, 1], f32)
    zero1_sb = sb.tile([1, 1], f32)
    dum_sb = sb.tile([1, 1], f32)

    m0_sb = cpk_sb[:, 0:256]
    eyef_sb = cpk_sb[:, 256:384]
    cb1_sb = cpk_sb[:, 384:388]
    b2c_sb = cpk_sb[:, 388:390]
    c1n_sb = cr1_sb[:, 0:H]
    c1on_sb = cr1_sb[:, H:H + D]

    mm = nc.tensor.matmul
    act = nc.scalar.activation
    tt_ = nc.vector.tensor_tensor
    tcp = nc.vector.tensor_copy
    ts_ = nc.vector.tensor_scalar
    stt = nc.vector.scalar_tensor_tensor

    # warmup source on the (idle) gpsimd queue so the PE starts ASAP
    nc.gpsimd.memset(wu_sb[:], 0.0)
    nc.vector.memset(halfpi_sb[:], PI / 2)
    nc.vector.memset(zerop_sb[:], 0.0)
    nc.vector.memset(zero1_sb[:], 0.0)
    nc.vector.memset(dum_sb[:], 0.0)
    nc.vector.memset(ones_sb[:].bitcast(f32), 1.0)
    nc.vector.memset(invdb_sb[:], 1.0 / D)

    # -------- DMA issues (sync + gpsimd only; scalar runs acts) --------
    # xpet/xpo columns are split across BOTH queues so the prefix chain
    # isn't gated by one queue's backlog.
    dma_s = nc.sync.dma_start
    dma_g = nc.gpsimd.dma_start
    dma_g(kwb_sb[:], io["kwb"])
    d_xt0 = dma_s(xt_sb[:, 0:512], io["xt"][:, 0:512])
    dma_g(vw_sb[:], io["vw"])
    d_xt1 = dma_s(xt_sb[:, 512:1024], io["xt"][:, 512:1024])
    dma_g(cpk_sb[:], io["cpk"])
    dma_s(xpet_sb[:, 0:384], io["xpetl"][:, 0:384])
    dma_g(xpet_sb[:, 768:1152], io["xpetl"][:, 384:768])
    dma_s(xpet_sb[:, 384:768], io["xpeth"][:, 0:384])
    dma_g(xpet_sb[:, 1152:1536], io["xpeth"][:, 384:768])
    dma_s(xpo_sb[:, 0:768], io["xpol"])
    dma_g(xpo_sb[:, 768:1536], io["xpoh"])
    dma_s(w2_sb[:], io["w2"])
    dma_g(w1g_sb[:], io["w1g"])
    dma_s(wog_sb[:], io["wog"])
    dma_g(cr1_sb[:], io["cr1"])
    dma_s(xc_sb[:], io["xc"])

    # ------------ PE warmup (release the HAM clock gate) ------------
    wu_ps = pt.tile([128, 512], f32, tag="pt")

    def warm(n):
        for i in range(n):
            mm(wu_ps[:], wu_sb[:, 0:128], wu_sb[:, 128:640],
               start=True, stop=True)

    warm(5)

    # prime the silu act table (covers tanh/sin/square) exactly once
    a_dum = act(dum_sb[:], dum_sb[:], DUM_AF, bias=zero1_sb[:])

    # ---------------- own phases -> Q ----------------
    ph_ps = pa.tile([P, T], f32, tag="pa")
    for kt in range(2):
        mm(ph_ps[:], kwb_sb[:, 32 * kt: 32 * kt + 32],
           xt_sb[:, 512 * kt: 512 * kt + 512],
           start=(kt == 0), stop=(kt == 1))
    warm(4)
    a_tanh = act(tv_sb[P:2 * P, :], ph_ps[:], AF.Tanh, bias=zerop_sb[0:P, :])
    add_dep_helper(a_tanh.ins, a_dum.ins, sync=True,
                   reason="act table: silu table covers tanh/sin/square")
    ts_(out=tv_sb[0:P, :].bitcast(i32), in0=tv_sb[P:2 * P, :].bitcast(i32),
        scalar1=0x7FFFFFFF, scalar2=None, op0=OP.bitwise_and)
    # cos(pi*t) = sin(pi/2 - pi*|t|); sin table argument stays in range
    act(qb_sb[0:P, :], tv_sb[0:P, :], AF.Sin,
        bias=halfpi_sb[0:P, :], scale=-PI)
    a_oq = act(qb_sb[P:2 * P, :], tv_sb[P:2 * P, :], AF.Sin,
               bias=zerop_sb[0:P, :], scale=PI)

    # ---------------- V at odd own tokens ----------------
    for blk in range(2):
        vp = pa.tile([128, D], f32, tag="pa")
        for kt in range(2):
            mm(vp[:], xt_sb[:, 512 * kt + 256 * blk + 1:
                            512 * kt + 256 * blk + 256: 2],
               vw_sb[:, 256 * kt: 256 * kt + 256],
               start=(kt == 0), stop=(kt == 1))
        tcp(vodd_sb[:, 256 * blk: 256 * blk + 256], vp[:])

    # ---------------- prefix phases (token-major) -> kpre --------------
    pp_ps = pa.tile([128, 192], f32, tag="pa")
    for j in range(6):
        for kt in range(2):
            mm(pp_ps[:, 32 * j: 32 * j + 32],
               xpet_sb[:, 768 * kt + 128 * j: 768 * kt + 128 * j + 128],
               kwb_sb[:, 32 * kt: 32 * kt + 32],
               start=(kt == 0), stop=(kt == 1))
    a_pt = act(tvp_sb[:, 0:192], pp_ps[:], AF.Tanh, bias=zerop_sb[:])
    add_dep_helper(a_pt.ins, a_oq.ins, sync=True,
                   reason="own qb before prefix acts on the scalar queue")
    ts_(out=tvp_sb[:, 192:384].bitcast(i32), in0=tvp_sb[:, 0:192].bitcast(i32),
        scalar1=0x7FFFFFFF, scalar2=None, op0=OP.bitwise_and)
    act(kcs_sb[:, 0:192], tvp_sb[:, 192:384], AF.Sin,
        bias=halfpi_sb[:], scale=-PI)
    act(kcs_sb[:, 192:384], tvp_sb[:, 0:192], AF.Sin,
        bias=zerop_sb[:], scale=PI)

    # ---------------- scores (odd tk only) + causal mask ----------------
    sc0 = pb.tile([128, 512], f32, tag="pb")
    mm(sc0[:], qb_sb[:, 0:255:2], qb_sb[:], start=True, stop=True)
    tt_(ss_sb[:, 0:256], sc0[:, 0:256], m0_sb[:], OP.mult)
    tcp(ss_sb[:, 256:512], sc0[:, 256:512])
    sc1 = pa.tile([128, 256], f32, tag="pa")
    mm(sc1[:], qb_sb[:, 256:511:2], qb_sb[:, 256:512], start=True, stop=True)
    tt_(ss_sb[:, 512:768], sc1[:], m0_sb[:], OP.mult)

    # ---------------- prefix G and S ----------------
    # cos/sin accumulation groups live in different PSUM banks (pt vs pa
    # pools) so they can interleave while sharing the xpo stationary.
    for dh in range(2):
        gpc = pt.tile([128, 32], f32, tag="pt")
        gps = pa.tile([128, 32], f32, tag="pa")
        for j in range(6):
            mm(gpc[:],
               xpo_sb[:, 256 * j + 128 * dh: 256 * j + 128 * dh + 128],
               kcs_sb[:, 32 * j: 32 * j + 32],
               start=(j == 0), stop=(j == 5))
            mm(gps[:],
               xpo_sb[:, 256 * j + 128 * dh: 256 * j + 128 * dh + 128],
               kcs_sb[:, 192 + 32 * j: 192 + 32 * j + 32],
               start=(j == 0), stop=(j == 5))
        tcp(gT_sb[:, 64 * dh: 64 * dh + 32], gpc[:])
        tcp(gT_sb[:, 64 * dh + 32: 64 * dh + 64], gps[:])
    s_ps = pa.tile([2 * P, D], f32, tag="pa")
    for kt in range(2):
        mm(s_ps[:], gT_sb[:, 64 * kt: 64 * kt + 64],
           vw_sb[:, 256 * kt: 256 * kt + 256],
           start=(kt == 0), stop=(kt == 1))
    tcp(s_sb[:], s_ps[:])

    # ------------- retrieved^T = tril(V^T ss) + S^T Q -------------
    retr = []
    for dh in range(2):
        rp = pb.tile([128, 512], f32, tag="pb")
        # one PSUM bank allows a single open accumulation group: close the
        # first-half group before opening the second-half one
        mm(rp[:, 0:256], vodd_sb[:, 128 * dh: 128 * dh + 128],
           ss_sb[:, 0:256], start=True, stop=False)
        mm(rp[:, 0:256], s_sb[:, 128 * dh: 128 * dh + 128],
           qb_sb[:, 0:256], start=False, stop=True)
        mm(rp[:, 256:512], vodd_sb[:, 128 * dh: 128 * dh + 128],
           ss_sb[:, 256:512], start=True, stop=False)
        mm(rp[:, 256:512], vodd_sb[:, 256 + 128 * dh: 256 + 128 * dh + 128],
           ss_sb[:, 512:768], start=False, stop=False)
        mm(rp[:, 256:512], s_sb[:, 128 * dh: 128 * dh + 128],
           qb_sb[:, 256:512], start=False, stop=True)
        retr.append(rp)

    # ------- LN1 stats: casts/squares split over DVE and scalar -------
    tcp(r_sb[:, 0:512], retr[0][:])                       # DVE cast
    a_c1 = act(r_sb[:, 512:1024], retr[1][:], AF.Identity,
               bias=zerop_sb[:])                          # scalar copy
    act(sq_sb[:, 0:512], retr[0][:], AF.Square, bias=zerop_sb[:])
    tt_(sq_sb[:, 512:1024], r_sb[:, 512:1024], r_sb[:, 512:1024], OP.mult)
    st1 = pa.tile([1, T], f32, tag="pa")
    ms1 = pa.tile([1, T], f32, tag="pa2", bufs=1)
    for kt in range(2):
        mm(st1[0:1, :], invdb_sb[:, 0:1], r_sb[:, 512 * kt: 512 * kt + 512],
           start=(kt == 0), stop=(kt == 1))
    for kt in range(2):
        mm(ms1[0:1, :], invdb_sb[:, 0:1], sq_sb[:, 512 * kt: 512 * kt + 512],
           start=(kt == 0), stop=(kt == 1))
    tcp(mean1_sb[:], st1[0:1, :])
    tt_(m2_sb[:], mean1_sb[:], mean1_sb[:], OP.mult)
    stt(out=var1_sb[:], in0=ms1[0:1, :], scalar=1e-5, in1=m2_sb[:],
        op0=OP.add, op1=OP.subtract)

    def ln_rstd(var_sb, stdc, rstdc, nwt):
        # var row -> columns [128,4]; rstd = rsqrt(var) via magic-seed
        # Newton (1 iter, fused to 3 DVE ops), float work on DVE.
        vc = pt.tile([128, 4], f32, tag="pt")
        for j in range(4):
            nc.tensor.transpose(vc[:, j: j + 1],
                                var_sb[0:1, 128 * j: 128 * j + 128],
                                eyef_sb[0:1, 0:1])
        tcp(stdc[:], vc[:])
        ts_(out=rstdc[:].bitcast(i32), in0=stdc[:].bitcast(i32), scalar1=1,
            scalar2=None, op0=OP.logical_shift_right)
        ts_(out=rstdc[:].bitcast(i32), in0=rstdc[:].bitcast(i32), scalar1=-1,
            scalar2=0x5F3759DF, op0=OP.mult, op1=OP.add)
        tt_(nwt[:], rstdc[:], rstdc[:], OP.mult)
        stt(out=nwt[:], in0=nwt[:], scalar=-0.5, in1=stdc[:],
            op0=OP.mult, op1=OP.mult)                  # -0.5*var*y^2
        stt(out=rstdc[:], in0=nwt[:], scalar=1.5, in1=rstdc[:],
            op0=OP.add, op1=OP.mult)                   # y*(1.5 - 0.5*var*y^2)

    # ---- W1 on raw r while the rstd chain runs on DVE (post-scale):
    #   (W1g^T r + c1n (x) mean1) * rstd1 == W1g^T(LN1(r))
    def w1_block(m):
        hp = pb.tile([128, 512], f32, tag="pb")
        for kt in range(2):
            mm(hp[:], w1g_sb[:, 512 * kt + 128 * m: 512 * kt + 128 * m + 128],
               r_sb[:, 512 * kt: 512 * kt + 512],
               start=(kt == 0), stop=False)
        mm(hp[:], c1n_sb[0:1, 128 * m: 128 * m + 128], mean1_sb[:],
           start=False, stop=True)
        return hp

    hps = [w1_block(0)]
    ln_rstd(var1_sb, stdc_sb, rstdc_sb, nwt1_sb)
    hps.append(w1_block(1))
    hps.append(w1_block(2))
    rr = pt.tile([1, T], f32, tag="pt")
    for j in range(4):
        nc.tensor.transpose(rr[0:1, 128 * j: 128 * j + 128],
                            rstdc_sb[:, j: j + 1], eyef_sb[:])
    tcp(rstd1_sb[:], rr[:])
    rb1 = pa.tile([128, 512], f32, tag="pa")
    mm(rb1[:], _r(ones_sb[:]), _r(rstd1_sb[:]), start=True, stop=True)
    tcp(rb1s_sb[:], rb1[:])
    hps.append(w1_block(3))
    for m in range(4):
        tt_(rs_sb[:, 512 * m: 512 * m + 512],
            hps[m][:], rb1s_sb[:], OP.mult)
        act(h_sb[:, 512 * m: 512 * m + 512],
            rs_sb[:, 512 * m: 512 * m + 512], GELU_AF,
            bias=cb1_sb[:, m: m + 1])

    # ---------------- W2 -> refined ----------------
    ref = []
    for dh in range(2):
        fp = pb.tile([128, 512], f32, tag="pb")
        for kt in range(4):
            mm(fp[:], w2_sb[:, 256 * kt + 128 * dh: 256 * kt + 128 * dh + 128],
               h_sb[:, 512 * kt: 512 * kt + 512],
               start=(kt == 0), stop=(kt == 3))
        ref.append(fp)

    # ------- LN2 stats (+b2 bias), split over DVE and scalar -------
    ts_(out=f_sb[:, 0:512], in0=ref[0][:], scalar1=b2c_sb[:, 0:1],
        scalar2=None, op0=OP.add)                          # DVE
    act(f_sb[:, 512:1024], ref[1][:], AF.Identity, bias=b2c_sb[:, 1:2])
    act(sq2_sb[:, 0:512], ref[0][:], AF.Square, bias=b2c_sb[:, 0:1])
    tt_(sq2_sb[:, 512:1024], f_sb[:, 512:1024], f_sb[:, 512:1024], OP.mult)
    st2 = pa.tile([1, T], f32, tag="pa")
    ms2 = pa.tile([1, T], f32, tag="pa2", bufs=1)
    for kt in range(2):
        mm(st2[0:1, :], invdb_sb[:, 0:1], f_sb[:, 512 * kt: 512 * kt + 512],
           start=(kt == 0), stop=(kt == 1))
    for kt in range(2):
        mm(ms2[0:1, :], invdb_sb[:, 0:1], sq2_sb[:, 512 * kt: 512 * kt + 512],
           start=(kt == 0), stop=(kt == 1))
    tcp(mean2_sb[:], st2[0:1, :])
    tt_(m2b_sb[:], mean2_sb[:], mean2_sb[:], OP.mult)
    stt(out=var2_sb[:], in0=ms2[0:1, :], scalar=1e-5, in1=m2b_sb[:],
        op0=OP.add, op1=OP.subtract)

    # Wo (token-major) + rank-1 mean fix; Newton for rstd2 runs on DVE
    # underneath these matmuls; rstd2 applies inside the residual op.
    ops = []
    for tm in range(4):
        op = pa.tile([128, D], f32, tag="pa")
        for dh in range(2):
            mm(op[:], f_sb[:, 512 * dh + 128 * tm: 512 * dh + 128 * tm + 128],
               wog_sb[:, 256 * dh: 256 * dh + 256],
               start=(dh == 0), stop=False)
        mm(op[:], mean2_sb[0:1, 128 * tm: 128 * tm + 128], c1on_sb[:],
           start=False, stop=True)
        ops.append(op)
        if tm == 0:
            ln_rstd(var2_sb, stdc2_sb, rstdc2_sb, nwt2_sb)
        if tm >= 1:
            t0 = tm - 1
            stt(out=out_sb[:, 256 * t0: 256 * t0 + 256], in0=ops[t0][:],
                scalar=rstdc2_sb[:, t0: t0 + 1],
                in1=xc_sb[:, 256 * t0: 256 * t0 + 256],
                op0=OP.mult, op1=OP.add)
            dq = dma_s if t0 % 2 == 0 else dma_g
            dq(outc[128 * t0: 128 * t0 + 128, :],
               out_sb[:, 256 * t0: 256 * t0 + 256])
    stt(out=out_sb[:, 768:1024], in0=ops[3][:],
        scalar=rstdc2_sb[:, 3:4], in1=xc_sb[:, 768:1024],
        op0=OP.mult, op1=OP.add)
    dma_g(outc[384:512, :], out_sb[:, 768:1024])

    pb.release()
    pa.release()
    pt.release()
    sb.release()


_CACHE = {}


def _get_nc():
    if "nc" not in _CACHE:
        _CACHE["nc"] = _build()
    return _CACHE["nc"]


def _bf(a):
    return np.asarray(a, np.float32).astype(ml_dtypes.bfloat16)


def _f16(a):
    return np.asarray(a, np.float32).astype(np.float16)


def _img(a, nblk, w):
    # [nblk*128, w] -> [128, nblk*w] SBUF image (block b at cols w*b)
    a = np.asarray(a, np.float32)
    return np.ascontiguousarray(
        a.reshape(nblk, 128, w).transpose(1, 0, 2).reshape(128, nblk * w))


def kernel(**inputs):
    x = np.asarray(inputs["x"], np.float32)
    key_W = np.asarray(inputs["key_W"], np.float32)
    key_b = np.asarray(inputs["key_b"], np.float32)
    val_W = np.asarray(inputs["val_W"], np.float32)
    val_b = np.asarray(inputs["val_b"], np.float32)
    ln1_g = np.asarray(inputs["ln1_g"], np.float32)
    ln1_b = np.asarray(inputs["ln1_b"], np.float32)
    W1 = np.asarray(inputs["W1"], np.float32)
    b1 = np.asarray(inputs["b1"], np.float32)
    W2 = np.asarray(inputs["W2"], np.float32)
    b2 = np.asarray(inputs["b2"], np.float32)
    ln2_g = np.asarray(inputs["ln2_g"], np.float32)
    ln2_b = np.asarray(inputs["ln2_b"], np.float32)
    Wo = np.asarray(inputs["Wo"], np.float32)
    bo = np.asarray(inputs["bo"], np.float32)

    # these are identically zero for this module; the kernel folds them out
    assert np.allclose(val_b, 0.0), "nonzero val_b unsupported"
    assert np.allclose(key_b, 0.0), "nonzero key_b unsupported"
    assert np.allclose(bo + ln2_b @ Wo, 0.0), "nonzero output bias unsupported"

    w1g = ln1_g[:, None] * W1
    wog = ln2_g[:, None] * Wo
    m0 = (np.arange(1, 256, 2)[:, None] <=
          np.arange(256)[None, :]).astype(np.float32)
    cpk = np.concatenate([
        m0, np.eye(128, dtype=np.float32),
        (b1 + ln1_b @ W1).reshape(128, 4, order="F"),
        b2.reshape(128, 2, order="F")], axis=1).astype(np.float32)
    cr1 = np.concatenate([-w1g.sum(0), -wog.sum(0)])[None, :]
    shared = {
        "kwb": _f16(_img(key_W, 2, P)),
        "vw": _f16(_img(val_W, 2, D)),
        "w1g": _bf(_img(w1g, 2, H)),
        "w2": _bf(_img(W2, 4, D)),
        "wog": _bf(_img(wog, 2, D)),
        "cpk": np.ascontiguousarray(cpk), "cr1": _bf(cr1),
    }
    in_maps = []
    for i in range(N_CORES):
        b, c = divmod(i, 4)
        l0 = c * T
        npairs = l0 // 2
        xpe = np.zeros((PRE, D), np.float32)
        xpo = np.zeros((PRE, D), np.float32)
        if npairs:
            xpe[:npairs] = x[b, 0:l0 - 1:2]
            xpo[:npairs] = x[b, 1:l0:2]
        xch = np.ascontiguousarray(x[b, l0:l0 + T])
        xpet_img = _img(np.ascontiguousarray(xpe.T), 2, PRE)
        xpo_img = _img(xpo, 6, D)
        in_maps.append({
            "xt": _f16(_img(xch.T, 2, T)),
            "xc": _f16(_img(xch, 4, D)),
            "xpetl": _f16(np.concatenate(
                [xpet_img[:, 0:384], xpet_img[:, 768:1152]], axis=1)),
            "xpeth": _f16(np.concatenate(
                [xpet_img[:, 384:768], xpet_img[:, 1152:1536]], axis=1)),
            "xpol": _f16(xpo_img[:, 0:768]),
            "xpoh": _f16(xpo_img[:, 768:1536]),
            **shared,
        })

    nc = _get_nc()
    res = run_bass_kernel_spmd(nc, in_maps, core_ids=list(range(N_CORES)),
                               **_CACHE.get("run_kwargs", {}))
    _CACHE["last_result"] = res
    out = np.empty((B, L, D), np.float32)
    for i in range(N_CORES):
        b, c = divmod(i, 4)
        out[b, c * T:(c + 1) * T] = res.results[i]["outc"]
    return out


# revision 13
# speedup vs baseline: 1.1264x; 1.0400x over previous
"""Trainium2 Bass kernel for nn_HardcodedKVMemoryBlock (8 NeuronCores).

Sharding: core i handles batch b=i//4, sequence chunk c=i%4 (512 tokens).
The (B,L,P,D) cumsum is restructured as causal linear attention:
    retrieved = tril(Q K^T) @ V + Q @ S_prefix
with Q=[cos,sin] phasors (L x 64), V = values at odd positions, and the
cross-chunk carry S_prefix = (K_even^T @ x_odd) @ val_W computed
redundantly per core from a zero-padded prefix (no collectives).
The 1/sqrt(valid*P) normalization cancels inside LayerNorm1 (scale
invariance); ln gains are folded into W1/Wo and means are applied as
rank-1 PE updates.

v4 notes:
- x and phasors in fp16 (8x tighter mantissa than bf16, same DMA bytes)
- inputs are host-prepped SBUF images (one packet per partition)
- NO DMA descriptors on the scalar queue (descriptor gen is ~650ns each
  and was starving the act-table load); consts packed into one tensor
- 9 warmup matmuls guarantee >3.4us of continuous PE busy so the HAM
  clock gate releases (1.2 -> 2.4 GHz) before the real stream
- prefix phases token-major (cheap narrow acts, kpre born j-major)
- LN1 applied by pre-scaling r with the rstd broadcast, so GELU reads
  the W1 PSUM directly (bias only) - no intermediate hi tile
- LN2 rstd Newton runs on DVE underneath the Wo matmuls; rstd2 is
  applied inside the residual scalar_tensor_tensor
- casts/squares for LN stats split across scalar and DVE queues
"""

import math
import numpy as np
import ml_dtypes

import concourse.bass as bass
import concourse.tile as tile
from concourse import bacc, mybir
from concourse.bass_utils import run_bass_kernel_spmd
from concourse.tile import add_dep_helper

PI = math.pi
B, L, D, P = 2, 2048, 256, 32
T = 512          # own tokens per core
H = 512          # MLP hidden
PRE = 768        # padded prefix pair count (max prefix 1536 tokens / 2)
N_CORES = 8

f32 = mybir.dt.float32
f32r = mybir.dt.float32r
bf16 = mybir.dt.bfloat16
f16 = mybir.dt.float16
i32 = mybir.dt.int32
AF = mybir.ActivationFunctionType
OP = mybir.AluOpType
DUM_AF = AF.Silu    # table-priming dummy (CoreSim debug overrides this)
GELU_AF = AF.Gelu


def _r(ap):
    return ap.bitcast(f32r)


def _build():
    nc = bacc.Bacc("TRN2", target_bir_lowering=False, debug=False,
                   num_devices=N_CORES)

    def din(name, shape, dt):
        return nc.dram_tensor(name, shape, dt, kind="ExternalInput").ap()

    xt = din("xt", [128, 1024], f16)      # x^T image: dh at cols 512*dh
    xc = din("xc", [128, 1024], f16)      # token-major: tt at cols 256*tt
    xpetl = din("xpetl", [128, 768], f16)   # prefix even^T cols 0:384 per kt
    xpeth = din("xpeth", [128, 768], f16)   # prefix even^T cols 384:768 per kt
    xpol = din("xpol", [128, 768], f16)     # prefix odd jb 0:3
    xpoh = din("xpoh", [128, 768], f16)     # prefix odd jb 3:6
    kwm = din("kwm", [128, 320], f16)     # kwb (64) | m0 (256)
    vw = din("vw", [128, 512], f16)       # kt at cols 256*kt
    w1g = din("w1g", [128, 1024], bf16)   # kt at cols 512*kt
    w2 = din("w2", [128, 1024], bf16)     # h-block kt at cols 256*kt
    wog = din("wog", [128, 512], bf16)    # dh at cols 256*dh
    cpk = din("cpk", [128, 134], f32)     # eyef | cb1 | b2c
    cr1 = din("cr1", [1, H + D], bf16)    # c1n | c1on
    outc = nc.dram_tensor("outc", [T, D], f32, kind="ExternalOutput").ap()

    with tile.TileContext(nc) as tc:
        _emit(tc, locals())
    nc.compile()
    return nc


def _emit(tc, io):
    nc = tc.nc
    outc = io["outc"]

    sb = tc.alloc_tile_pool(name="sb", bufs=1)
    pt = tc.alloc_tile_pool(name="pt", bufs=1, space="PSUM")    # small
    pa = tc.alloc_tile_pool(name="pa", bufs=2, space="PSUM")    # narrow
    pb = tc.alloc_tile_pool(name="pb", bufs=3, space="PSUM")    # [128,512]

    # ---------------- SBUF tiles ----------------
    wu_sb = sb.tile([128, 640], bf16)         # warmup stationary+moving
    kwm_sb = sb.tile([128, 320], f16)
    vw_sb = sb.tile([128, 512], f16)
    w1g_sb = sb.tile([128, 1024], bf16)
    w2_sb = sb.tile([128, 1024], bf16)
    wog_sb = sb.tile([128, 512], bf16)
    cpk_sb = sb.tile([128, 134], f32)
    cr1_sb = sb.tile([1, H + D], bf16)
    xt_sb = sb.tile([128, 1024], f16)
    xc_sb = sb.tile([128, 1024], f16)
    xpet_sb = sb.tile([128, 1536], f16)
    xpo_sb = sb.tile([128, 1536], f16)

    tv_sb = sb.tile([64, 512], f32)           # rows 0:32 |t|, 32:64 tanh
    qb_sb = sb.tile([64, 512], f16)           # rows 0:32 cos, 32:64 sin
    tvp_sb = sb.tile([128, 384], f32)         # cols 0:192 tanh, 192:384 abs
    kcs_sb = sb.tile([128, 384], f16)         # cols 0:192 cos, 192:384 sin
    vodd_sb = sb.tile([128, 512], f16)        # blk at cols 256*blk
    gT_sb = sb.tile([128, 128], f16)          # dh at cols 64*dh
    s_sb = sb.tile([64, 256], f16)
    ss_sb = sb.tile([128, 768], f16)          # ss0 (512) | ss1 (256)
    r_sb = sb.tile([128, 1024], bf16)         # dh at cols 512*dh
    sq_sb = sb.tile([128, 1024], bf16)
    rs_sb = sb.tile([128, 2048], f16)         # hi staging (hp * rstd1)
    rb1s_sb = sb.tile([128, 512], f32)
    h_sb = sb.tile([128, 2048], bf16)         # m at cols 512*m
    f_sb = sb.tile([128, 1024], bf16)
    sq2_sb = sb.tile([128, 1024], bf16)
    out_sb = sb.tile([128, 1024], f32)        # tt at cols 256*tt

    mean1_sb = sb.tile([1, T], bf16)
    var1_sb = sb.tile([1, T], f32)
    m2_sb = sb.tile([1, T], f32)
    rstd1_sb = sb.tile([1, T], f32r)
    mean2_sb = sb.tile([1, T], bf16)
    var2_sb = sb.tile([1, T], f32)
    m2b_sb = sb.tile([1, T], f32)
    stdc_sb = sb.tile([128, 4], f32)
    rstdc_sb = sb.tile([128, 4], f32)
    nwt1_sb = sb.tile([128, 4], f32)
    stdc2_sb = sb.tile([128, 4], f32)
    rstdc2_sb = sb.tile([128, 4], f32)
    nwt2_sb = sb.tile([128, 4], f32)

    ones_sb = sb.tile([1, 128], f32r)
    invdb_sb = sb.tile([128, 1], bf16)
    halfpi_sb = sb.tile([128, 1], f32)
    zerop_sb = sb.tile([128, 1], f32)
    zero1_sb = sb.tile([1, 1], f32)
    dum_sb = sb.tile([1, 1], f32)

    kwb_sb = kwm_sb[:, 0:64]
    m0_sb = kwm_sb[:, 64:320]
    eyef_sb = cpk_sb[:, 0:128]
    cb1_sb = cpk_sb[:, 128:132]
    b2c_sb = cpk_sb[:, 132:134]
    c1n_sb = cr1_sb[:, 0:H]
    c1on_sb = cr1_sb[:, H:H + D]

    mm = nc.tensor.matmul
    act = nc.scalar.activation
    tt_ = nc.vector.tensor_tensor
    tcp = nc.vector.tensor_copy
    ts_ = nc.vector.tensor_scalar
    stt = nc.vector.scalar_tensor_tensor

    # warmup source on the (idle) gpsimd queue so the PE starts ASAP
    nc.gpsimd.memset(wu_sb[:], 0.0)
    nc.vector.memset(halfpi_sb[:], PI / 2)
    nc.vector.memset(zerop_sb[:], 0.0)
    nc.vector.memset(zero1_sb[:], 0.0)
    nc.vector.memset(dum_sb[:], 0.0)
    nc.vector.memset(ones_sb[:].bitcast(f32), 1.0)
    nc.vector.memset(invdb_sb[:], 1.0 / D)

    # -------- DMA issues (sync + gpsimd only; scalar runs acts) --------
    # xpet/xpo columns are split across BOTH queues so the prefix chain
    # isn't gated by one queue's backlog.
    dma_s = nc.sync.dma_start
    dma_g = nc.gpsimd.dma_start
    dma_g(kwm_sb[:], io["kwm"])
    d_xt0 = dma_s(xt_sb[:, 0:512], io["xt"][:, 0:512])
    dma_g(vw_sb[:], io["vw"])
    d_xt1 = dma_s(xt_sb[:, 512:1024], io["xt"][:, 512:1024])
    dma_g(xpet_sb[:, 768:1152], io["xpetl"][:, 384:768])
    dma_s(xpet_sb[:, 0:384], io["xpetl"][:, 0:384])
    dma_g(xpet_sb[:, 1152:1536], io["xpeth"][:, 384:768])
    dma_s(xpet_sb[:, 384:768], io["xpeth"][:, 0:384])
    dma_g(xpo_sb[:, 768:1536], io["xpoh"])
    dma_s(xpo_sb[:, 0:768], io["xpol"])
    dma_g(cpk_sb[:], io["cpk"])
    dma_s(w1g_sb[:], io["w1g"])
    dma_g(xc_sb[:], io["xc"])
    dma_s(w2_sb[:], io["w2"])
    dma_g(cr1_sb[:], io["cr1"])
    dma_s(wog_sb[:], io["wog"])

    # ------------ PE warmup (release the HAM clock gate) ------------
    wu_ps = pt.tile([128, 512], f32, tag="pt")

    def warm(n):
        for i in range(n):
            mm(wu_ps[:], wu_sb[:, 0:128], wu_sb[:, 128:640],
               start=True, stop=True)

    warm(5)

    # prime the silu act table (covers tanh/sin/square) exactly once
    a_dum = act(dum_sb[:], dum_sb[:], DUM_AF, bias=zero1_sb[:])

    # ---------------- own phases -> Q ----------------
    ph_ps = pa.tile([P, T], f32, tag="pa")
    for kt in range(2):
        mm(ph_ps[:], kwb_sb[:, 32 * kt: 32 * kt + 32],
           xt_sb[:, 512 * kt: 512 * kt + 512],
           start=(kt == 0), stop=(kt == 1))
    warm(4)
    a_tanh = act(tv_sb[P:2 * P, :], ph_ps[:], AF.Tanh, bias=zerop_sb[0:P, :])
    add_dep_helper(a_tanh.ins, a_dum.ins, sync=True,
                   reason="act table: silu table covers tanh/sin/square")
    ts_(out=tv_sb[0:P, :].bitcast(i32), in0=tv_sb[P:2 * P, :].bitcast(i32),
        scalar1=0x7FFFFFFF, scalar2=None, op0=OP.bitwise_and)
    # cos(pi*t) = sin(pi/2 - pi*|t|); sin table argument stays in range
    act(qb_sb[0:P, :], tv_sb[0:P, :], AF.Sin,
        bias=halfpi_sb[0:P, :], scale=-PI)
    a_oq = act(qb_sb[P:2 * P, :], tv_sb[P:2 * P, :], AF.Sin,
               bias=zerop_sb[0:P, :], scale=PI)

    # ---------------- V at odd own tokens ----------------
    for blk in range(2):
        vp = pa.tile([128, D], f32, tag="pa")
        for kt in range(2):
            mm(vp[:], xt_sb[:, 512 * kt + 256 * blk + 1:
                            512 * kt + 256 * blk + 256: 2],
               vw_sb[:, 256 * kt: 256 * kt + 256],
               start=(kt == 0), stop=(kt == 1))
        tcp(vodd_sb[:, 256 * blk: 256 * blk + 256], vp[:])

    # ---------------- scores (odd tk only) + causal mask ----------------
    sc0 = pb.tile([128, 512], f32, tag="pb")
    mm(sc0[:], qb_sb[:, 0:255:2], qb_sb[:], start=True, stop=True)
    tt_(ss_sb[:, 0:256], sc0[:, 0:256], m0_sb[:], OP.mult)
    tcp(ss_sb[:, 256:512], sc0[:, 256:512])
    sc1 = pa.tile([128, 256], f32, tag="pa")
    mm(sc1[:], qb_sb[:, 256:511:2], qb_sb[:, 256:512], start=True, stop=True)
    tt_(ss_sb[:, 512:768], sc1[:], m0_sb[:], OP.mult)

    # ---------------- prefix phases (token-major) -> kpre --------------
    pp_ps = pa.tile([128, 192], f32, tag="pa")
    for j in range(6):
        for kt in range(2):
            mm(pp_ps[:, 32 * j: 32 * j + 32],
               xpet_sb[:, 768 * kt + 128 * j: 768 * kt + 128 * j + 128],
               kwb_sb[:, 32 * kt: 32 * kt + 32],
               start=(kt == 0), stop=(kt == 1))
    a_pt = act(tvp_sb[:, 0:192], pp_ps[:], AF.Tanh, bias=zerop_sb[:])
    add_dep_helper(a_pt.ins, a_oq.ins, sync=True,
                   reason="own qb before prefix acts on the scalar queue")
    ts_(out=tvp_sb[:, 192:384].bitcast(i32), in0=tvp_sb[:, 0:192].bitcast(i32),
        scalar1=0x7FFFFFFF, scalar2=None, op0=OP.bitwise_and)
    act(kcs_sb[:, 0:192], tvp_sb[:, 192:384], AF.Sin,
        bias=halfpi_sb[:], scale=-PI)
    act(kcs_sb[:, 192:384], tvp_sb[:, 0:192], AF.Sin,
        bias=zerop_sb[:], scale=PI)

    # ---------------- prefix G and S ----------------
    # cos/sin accumulation groups live in different PSUM banks (pt vs pa
    # pools) so they can interleave while sharing the xpo stationary.
    for dh in range(2):
        gpc = pt.tile([128, 32], f32, tag="pt")
        gps = pa.tile([128, 32], f32, tag="pa")
        for j in range(6):
            mm(gpc[:],
               xpo_sb[:, 256 * j + 128 * dh: 256 * j + 128 * dh + 128],
               kcs_sb[:, 32 * j: 32 * j + 32],
               start=(j == 0), stop=(j == 5))
            mm(gps[:],
               xpo_sb[:, 256 * j + 128 * dh: 256 * j + 128 * dh + 128],
               kcs_sb[:, 192 + 32 * j: 192 + 32 * j + 32],
               start=(j == 0), stop=(j == 5))
        tcp(gT_sb[:, 64 * dh: 64 * dh + 32], gpc[:])
        tcp(gT_sb[:, 64 * dh + 32: 64 * dh + 64], gps[:])
    s_ps = pa.tile([2 * P, D], f32, tag="pa")
    for kt in range(2):
        mm(s_ps[:], gT_sb[:, 64 * kt: 64 * kt + 64],
           vw_sb[:, 256 * kt: 256 * kt + 256],
           start=(kt == 0), stop=(kt == 1))
    tcp(s_sb[:], s_ps[:])

    # ------------- retrieved^T = tril(V^T ss) + S^T Q -------------
    retr = []
    for dh in range(2):
        rp = pb.tile([128, 512], f32, tag="pb")
        # one PSUM bank allows a single open accumulation group: close the
        # first-half group before opening the second-half one
        mm(rp[:, 0:256], vodd_sb[:, 128 * dh: 128 * dh + 128],
           ss_sb[:, 0:256], start=True, stop=False)
        mm(rp[:, 0:256], s_sb[:, 128 * dh: 128 * dh + 128],
           qb_sb[:, 0:256], start=False, stop=True)
        mm(rp[:, 256:512], vodd_sb[:, 128 * dh: 128 * dh + 128],
           ss_sb[:, 256:512], start=True, stop=False)
        mm(rp[:, 256:512], vodd_sb[:, 256 + 128 * dh: 256 + 128 * dh + 128],
           ss_sb[:, 512:768], start=False, stop=False)
        mm(rp[:, 256:512], s_sb[:, 128 * dh: 128 * dh + 128],
           qb_sb[:, 256:512], start=False, stop=True)
        retr.append(rp)

    # ------- LN1 stats: casts/squares split over DVE and scalar -------
    tcp(r_sb[:, 0:512], retr[0][:])                       # DVE cast
    a_c1 = act(r_sb[:, 512:1024], retr[1][:], AF.Identity,
               bias=zerop_sb[:])                          # scalar copy
    act(sq_sb[:, 0:512], retr[0][:], AF.Square, bias=zerop_sb[:])
    tt_(sq_sb[:, 512:1024], r_sb[:, 512:1024], r_sb[:, 512:1024], OP.mult)
    st1 = pa.tile([1, T], f32, tag="pa")
    ms1 = pa.tile([1, T], f32, tag="pa2", bufs=1)
    for kt in range(2):
        mm(st1[0:1, :], invdb_sb[:, 0:1], r_sb[:, 512 * kt: 512 * kt + 512],
           start=(kt == 0), stop=(kt == 1))
    for kt in range(2):
        mm(ms1[0:1, :], invdb_sb[:, 0:1], sq_sb[:, 512 * kt: 512 * kt + 512],
           start=(kt == 0), stop=(kt == 1))
    tcp(mean1_sb[:], st1[0:1, :])
    tt_(m2_sb[:], mean1_sb[:], mean1_sb[:], OP.mult)
    stt(out=var1_sb[:], in0=ms1[0:1, :], scalar=1e-5, in1=m2_sb[:],
        op0=OP.add, op1=OP.subtract)

    def ln_rstd(var_sb, stdc, rstdc, nwt):
        # var row -> columns [128,4]; rstd = rsqrt(var) via magic-seed
        # Newton (1 iter, fused to 3 DVE ops), float work on DVE.
        vc = pt.tile([128, 4], f32, tag="pt")
        for j in range(4):
            nc.tensor.transpose(vc[:, j: j + 1],
                                var_sb[0:1, 128 * j: 128 * j + 128],
                                eyef_sb[0:1, 0:1])
        tcp(stdc[:], vc[:])
        ts_(out=rstdc[:].bitcast(i32), in0=stdc[:].bitcast(i32), scalar1=1,
            scalar2=None, op0=OP.logical_shift_right)
        ts_(out=rstdc[:].bitcast(i32), in0=rstdc[:].bitcast(i32), scalar1=-1,
            scalar2=0x5F3759DF, op0=OP.mult, op1=OP.add)
        tt_(nwt[:], rstdc[:], rstdc[:], OP.mult)
        stt(out=nwt[:], in0=nwt[:], scalar=-0.5, in1=stdc[:],
            op0=OP.mult, op1=OP.mult)                  # -0.5*var*y^2
        stt(out=rstdc[:], in0=nwt[:], scalar=1.5, in1=rstdc[:],
            op0=OP.add, op1=OP.mult)                   # y*(1.5 - 0.5*var*y^2)

    # ---- W1 on raw r while the rstd chain runs on DVE (post-scale):
    #   (W1g^T r + c1n (x) mean1) * rstd1 == W1g^T(LN1(r))
    def w1_block(m):
        hp = pb.tile([128, 512], f32, tag="pb")
        for kt in range(2):
            mm(hp[:], w1g_sb[:, 512 * kt + 128 * m: 512 * kt + 128 * m + 128],
               r_sb[:, 512 * kt: 512 * kt + 512],
               start=(kt == 0), stop=False)
        mm(hp[:], c1n_sb[0:1, 128 * m: 128 * m + 128], mean1_sb[:],
           start=False, stop=True)
        return hp

    hps = [w1_block(0)]
    ln_rstd(var1_sb, stdc_sb, rstdc_sb, nwt1_sb)
    hps.append(w1_block(1))
    hps.append(w1_block(2))
    rr = pt.tile([1, T], f32, tag="pt")
    for j in range(4):
        nc.tensor.transpose(rr[0:1, 128 * j: 128 * j + 128],
                            rstdc_sb[:, j: j + 1], eyef_sb[:])
    tcp(rstd1_sb[:], rr[:])
    rb1 = pa.tile([128, 512], f32, tag="pa")
    mm(rb1[:], _r(ones_sb[:]), _r(rstd1_sb[:]), start=True, stop=True)
    tcp(rb1s_sb[:], rb1[:])
    hps.append(w1_block(3))
    for m in range(4):
        tt_(rs_sb[:, 512 * m: 512 * m + 512],
            hps[m][:], rb1s_sb[:], OP.mult)
        act(h_sb[:, 512 * m: 512 * m + 512],
            rs_sb[:, 512 * m: 512 * m + 512], GELU_AF,
            bias=cb1_sb[:, m: m + 1])

    # ---------------- W2 -> refined ----------------
    ref = []
    for dh in range(2):
        fp = pb.tile([128, 512], f32, tag="pb")
        for kt in range(4):
            mm(fp[:], w2_sb[:, 256 * kt + 128 * dh: 256 * kt + 128 * dh + 128],
               h_sb[:, 512 * kt: 512 * kt + 512],
               start=(kt == 0), stop=(kt == 3))
        ref.append(fp)

    # ------- LN2 stats (+b2 bias), split over DVE and scalar -------
    ts_(out=f_sb[:, 0:512], in0=ref[0][:], scalar1=b2c_sb[:, 0:1],
        scalar2=None, op0=OP.add)                          # DVE
    act(f_sb[:, 512:1024], ref[1][:], AF.Identity, bias=b2c_sb[:, 1:2])
    act(sq2_sb[:, 0:512], ref[0][:], AF.Square, bias=b2c_sb[:, 0:1])
    tt_(sq2_sb[:, 512:1024], f_sb[:, 512:1024], f_sb[:, 512:1024], OP.mult)
    st2 = pa.tile([1, T], f32, tag="pa")
    ms2 = pa.tile([1, T], f32, tag="pa2", bufs=1)
    for kt in range(2):
        mm(st2[0:1, :], invdb_sb[:, 0:1], f_sb[:, 512 * kt: 512 * kt + 512],
           start=(kt == 0), stop=(kt == 1))
    for kt in range(2):
        mm(ms2[0:1, :], invdb_sb[:, 0:1], sq2_sb[:, 512 * kt: 512 * kt + 512],
           start=(kt == 0), stop=(kt == 1))
    tcp(mean2_sb[:], st2[0:1, :])
    tt_(m2b_sb[:], mean2_sb[:], mean2_sb[:], OP.mult)
    stt(out=var2_sb[:], in0=ms2[0:1, :], scalar=1e-5, in1=m2b_sb[:],
        op0=OP.add, op1=OP.subtract)

    # Wo (token-major) + rank-1 mean fix; Newton for rstd2 runs on DVE
    # underneath these matmuls; rstd2 applies inside the residual op.
    ops = []
    for tm in range(4):
        op = pa.tile([128, D], f32, tag="pa")
        for dh in range(2):
            mm(op[:], f_sb[:, 512 * dh + 128 * tm: 512 * dh + 128 * tm + 128],
               wog_sb[:, 256 * dh: 256 * dh + 256],
               start=(dh == 0), stop=False)
        mm(op[:], mean2_sb[0:1, 128 * tm: 128 * tm + 128], c1on_sb[:],
           start=False, stop=True)
        ops.append(op)
        if tm == 0:
            ln_rstd(var2_sb, stdc2_sb, rstdc2_sb, nwt2_sb)
        if tm >= 1:
            t0 = tm - 1
            stt(out=out_sb[:, 256 * t0: 256 * t0 + 256], in0=ops[t0][:],
                scalar=rstdc2_sb[:, t0: t0 + 1],
                in1=xc_sb[:, 256 * t0: 256 * t0 + 256],
                op0=OP.mult, op1=OP.add)
            dq = dma_s if t0 % 2 == 0 else dma_g
            dq(outc[128 * t0: 128 * t0 + 128, :],
               out_sb[:, 256 * t0: 256 * t0 + 256])
    stt(out=out_sb[:, 768:1024], in0=ops[3][:],
        scalar=rstdc2_sb[:, 3:4], in1=xc_sb[:, 768:1024],
        op0=OP.mult, op1=OP.add)
    dma_g(outc[384:512, :], out_sb[:, 768:1024])

    pb.release()
    pa.release()
    pt.release()
    sb.release()


_CACHE = {}


def _get_nc():
    if "nc" not in _CACHE:
        _CACHE["nc"] = _build()
    return _CACHE["nc"]


def _bf(a):
    return np.asarray(a, np.float32).astype(ml_dtypes.bfloat16)


def _f16(a):
    return np.asarray(a, np.float32).astype(np.float16)


def _img(a, nblk, w):
    # [nblk*128, w] -> [128, nblk*w] SBUF image (block b at cols w*b)
    a = np.asarray(a, np.float32)
    return np.ascontiguousarray(
        a.reshape(nblk, 128, w).transpose(1, 0, 2).reshape(128, nblk * w))


def kernel(**inputs):
    x = np.asarray(inputs["x"], np.float32)
    key_W = np.asarray(inputs["key_W"], np.float32)
    key_b = np.asarray(inputs["key_b"], np.float32)
    val_W = np.asarray(inputs["val_W"], np.float32)
    val_b = np.asarray(inputs["val_b"], np.float32)
    ln1_g = np.asarray(inputs["ln1_g"], np.float32)
    ln1_b = np.asarray(inputs["ln1_b"], np.float32)
    W1 = np.asarray(inputs["W1"], np.float32)
    b1 = np.asarray(inputs["b1"], np.float32)
    W2 = np.asarray(inputs["W2"], np.float32)
    b2 = np.asarray(inputs["b2"], np.float32)
    ln2_g = np.asarray(inputs["ln2_g"], np.float32)
    ln2_b = np.asarray(inputs["ln2_b"], np.float32)
    Wo = np.asarray(inputs["Wo"], np.float32)
    bo = np.asarray(inputs["bo"], np.float32)

    # these are identically zero for this module; the kernel folds them out
    assert np.allclose(val_b, 0.0), "nonzero val_b unsupported"
    assert np.allclose(key_b, 0.0), "nonzero key_b unsupported"
    assert np.allclose(bo + ln2_b @ Wo, 0.0), "nonzero output bias unsupported"

    w1g = ln1_g[:, None] * W1
    wog = ln2_g[:, None] * Wo
    m0 = (np.arange(1, 256, 2)[:, None] <=
          np.arange(256)[None, :]).astype(np.float32)
    kwm = np.concatenate([_img(key_W, 2, P), m0], axis=1)
    cpk = np.concatenate([
        np.eye(128, dtype=np.float32),
        (b1 + ln1_b @ W1).reshape(128, 4, order="F"),
        b2.reshape(128, 2, order="F")], axis=1).astype(np.float32)
    cr1 = np.concatenate([-w1g.sum(0), -wog.sum(0)])[None, :]
    shared = {
        "kwm": _f16(kwm),
        "vw": _f16(_img(val_W, 2, D)),
        "w1g": _bf(_img(w1g, 2, H)),
        "w2": _bf(_img(W2, 4, D)),
        "wog": _bf(_img(wog, 2, D)),
        "cpk": np.ascontiguousarray(cpk), "cr1": _bf(cr1),
    }
    in_maps = []
    for i in range(N_CORES):
        b, c = divmod(i, 4)
        l0 = c * T
        npairs = l0 // 2
        xpe = np.zeros((PRE, D), np.float32)
        xpo = np.zeros((PRE, D), np.float32)
        if npairs:
            xpe[:npairs] = x[b, 0:l0 - 1:2]
            xpo[:npairs] = x[b, 1:l0:2]
        xch = np.ascontiguousarray(x[b, l0:l0 + T])
        xpet_img = _img(np.ascontiguousarray(xpe.T), 2, PRE)
        xpo_img = _img(xpo, 6, D)
        in_maps.append({
            "xt": _f16(_img(xch.T, 2, T)),
            "xc": _f16(_img(xch, 4, D)),
            "xpetl": _f16(np.concatenate(
                [xpet_img[:, 0:384], xpet_img[:, 768:1152]], axis=1)),
            "xpeth": _f16(np.concatenate(
                [xpet_img[:, 384:768], xpet_img[:, 1152:1536]], axis=1)),
            "xpol": _f16(xpo_img[:, 0:768]),
            "xpoh": _f16(xpo_img[:, 768:1536]),
            **shared,
        })

    nc = _get_nc()
    res = run_bass_kernel_spmd(nc, in_maps, core_ids=list(range(N_CORES)),
                               **_CACHE.get("run_kwargs", {}))
    _CACHE["last_result"] = res
    out = np.empty((B, L, D), np.float32)
    for i in range(N_CORES):
        b, c = divmod(i, 4)
        out[b, c * T:(c + 1) * T] = res.results[i]["outc"]
    return out


# revision 16
# speedup vs baseline: 1.1494x; 1.0204x over previous
"""Trainium2 Bass kernel for nn_HardcodedKVMemoryBlock (8 NeuronCores).

Sharding: core i handles batch b=i//4, sequence chunk c=i%4 (512 tokens).
The (B,L,P,D) cumsum is restructured as causal linear attention:
    retrieved = tril(Q K^T) @ V + Q @ S_prefix
with Q=[cos,sin] phasors (L x 64), V = values at odd positions, and the
cross-chunk carry S_prefix = (K_even^T @ x_odd) @ val_W computed
redundantly per core from a zero-padded prefix (no collectives).
The 1/sqrt(valid*P) normalization cancels inside LayerNorm1 (scale
invariance); ln gains are folded into W1/Wo and means are applied as
rank-1 PE updates.

v4 notes:
- x and phasors in fp16 (8x tighter mantissa than bf16, same DMA bytes)
- inputs are host-prepped SBUF images (one packet per partition)
- NO DMA descriptors on the scalar queue (descriptor gen is ~650ns each
  and was starving the act-table load); consts packed into one tensor
- 9 warmup matmuls guarantee >3.4us of continuous PE busy so the HAM
  clock gate releases (1.2 -> 2.4 GHz) before the real stream
- prefix phases token-major (cheap narrow acts, kpre born j-major)
- LN1 applied by pre-scaling r with the rstd broadcast, so GELU reads
  the W1 PSUM directly (bias only) - no intermediate hi tile
- LN2 rstd Newton runs on DVE underneath the Wo matmuls; rstd2 is
  applied inside the residual scalar_tensor_tensor
- casts/squares for LN stats split across scalar and DVE queues
"""

import math
import numpy as np
import ml_dtypes

import concourse.bass as bass
import concourse.tile as tile
from concourse import bacc, mybir
from concourse.bass_utils import run_bass_kernel_spmd
from concourse.tile import add_dep_helper

PI = math.pi
B, L, D, P = 2, 2048, 256, 32
T = 512          # own tokens per core
H = 512          # MLP hidden
PRE = 768        # padded prefix pair count (max prefix 1536 tokens / 2)
N_CORES = 8

f32 = mybir.dt.float32
f32r = mybir.dt.float32r
bf16 = mybir.dt.bfloat16
f16 = mybir.dt.float16
i32 = mybir.dt.int32
AF = mybir.ActivationFunctionType
OP = mybir.AluOpType
DUM_AF = AF.Silu    # table-priming dummy (CoreSim debug overrides this)
GELU_AF = AF.Gelu


def _r(ap):
    return ap.bitcast(f32r)


def _build():
    nc = bacc.Bacc("TRN2", target_bir_lowering=False, debug=False,
                   num_devices=N_CORES)

    def din(name, shape, dt):
        return nc.dram_tensor(name, shape, dt, kind="ExternalInput").ap()

    xt = din("xt", [128, 1024], f16)      # x^T image: dh at cols 512*dh
    xc = din("xc", [128, 1024], f16)      # token-major: tt at cols 256*tt
    xpetl = din("xpetl", [128, 768], f16)   # prefix even^T cols 0:384 per kt
    xpeth = din("xpeth", [128, 768], f16)   # prefix even^T cols 384:768 per kt
    xpol = din("xpol", [128, 768], f16)     # prefix odd jb 0:3
    xpoh = din("xpoh", [128, 768], f16)     # prefix odd jb 3:6
    kwm = din("kwm", [128, 320], f16)     # kwb (64) | m0 (256)
    vw = din("vw", [128, 512], f16)       # kt at cols 256*kt
    w1g = din("w1g", [128, 1024], bf16)   # kt at cols 512*kt
    w2 = din("w2", [128, 1024], bf16)     # h-block kt at cols 256*kt
    wog = din("wog", [128, 512], bf16)    # dh at cols 256*dh
    cpk = din("cpk", [128, 134], f32)     # eyef | cb1 | b2c
    cr1 = din("cr1", [1, H + D], bf16)    # c1n | c1on
    outc = nc.dram_tensor("outc", [T, D], f32, kind="ExternalOutput").ap()

    with tile.TileContext(nc) as tc:
        _emit(tc, locals())
    nc.compile()
    return nc


def _emit(tc, io):
    nc = tc.nc
    outc = io["outc"]

    sb = tc.alloc_tile_pool(name="sb", bufs=1)
    pt = tc.alloc_tile_pool(name="pt", bufs=1, space="PSUM")    # small
    pa = tc.alloc_tile_pool(name="pa", bufs=2, space="PSUM")    # narrow
    pb = tc.alloc_tile_pool(name="pb", bufs=3, space="PSUM")    # [128,512]

    # ---------------- SBUF tiles ----------------
    wu_sb = sb.tile([128, 640], bf16)         # warmup stationary+moving
    kwm_sb = sb.tile([128, 320], f16)
    vw_sb = sb.tile([128, 512], f16)
    w1g_sb = sb.tile([128, 1024], bf16)
    w2_sb = sb.tile([128, 1024], bf16)
    wog_sb = sb.tile([128, 512], bf16)
    cpk_sb = sb.tile([128, 134], f32)
    cr1_sb = sb.tile([1, H + D], bf16)
    xt_sb = sb.tile([128, 1024], f16)
    xc_sb = sb.tile([128, 1024], f16)
    xpet_sb = sb.tile([128, 1536], f16)
    xpo_sb = sb.tile([128, 1536], f16)

    tv_sb = sb.tile([64, 512], f32)           # rows 0:32 |t|, 32:64 tanh
    qb_sb = sb.tile([64, 512], f16)           # rows 0:32 cos, 32:64 sin
    tvp_sb = sb.tile([128, 384], f32)         # cols 0:192 tanh, 192:384 abs
    kcs_sb = sb.tile([128, 384], f16)         # cols 0:192 cos, 192:384 sin
    vodd_sb = sb.tile([128, 512], f16)        # blk at cols 256*blk
    gT_sb = sb.tile([128, 128], f16)          # dh at cols 64*dh
    s_sb = sb.tile([64, 256], f16)
    ss_sb = sb.tile([128, 768], f16)          # ss0 (512) | ss1 (256)
    r_sb = sb.tile([128, 1024], bf16)         # dh at cols 512*dh
    sq_sb = sb.tile([128, 1024], bf16)
    rs_sb = sb.tile([128, 2048], f16)         # hi staging (hp * rstd1)
    rb1s_sb = sb.tile([128, 512], f32)
    h_sb = sb.tile([128, 2048], bf16)         # m at cols 512*m
    f_sb = sb.tile([128, 1024], bf16)
    sq2_sb = sb.tile([128, 1024], bf16)
    out_sb = sb.tile([128, 1024], f32)        # tt at cols 256*tt

    mean1_sb = sb.tile([1, T], bf16)
    var1_sb = sb.tile([1, T], f32)
    m2_sb = sb.tile([1, T], f32)
    rstd1_sb = sb.tile([1, T], f32)
    rstd1h_sb = sb.tile([1, T], f16)
    mean2_sb = sb.tile([1, T], bf16)
    var2_sb = sb.tile([1, T], f32)
    m2b_sb = sb.tile([1, T], f32)
    std1_sb = sb.tile([1, T], f32)
    stdc2_sb = sb.tile([128, 4], f32)
    rstdc2_sb = sb.tile([128, 4], f32)
    nwt2_sb = sb.tile([128, 4], f32)

    ones_sb = sb.tile([1, 128], f16)
    invdb_sb = sb.tile([128, 1], bf16)
    halfpi_sb = sb.tile([128, 1], f32)
    zerop_sb = sb.tile([128, 1], f32)
    zero1_sb = sb.tile([1, 1], f32)
    dum_sb = sb.tile([1, 1], f32)

    kwb_sb = kwm_sb[:, 0:64]
    m0_sb = kwm_sb[:, 64:320]
    eyef_sb = cpk_sb[:, 0:128]
    cb1_sb = cpk_sb[:, 128:132]
    b2c_sb = cpk_sb[:, 132:134]
    c1n_sb = cr1_sb[:, 0:H]
    c1on_sb = cr1_sb[:, H:H + D]

    mm = nc.tensor.matmul
    act = nc.scalar.activation
    tt_ = nc.vector.tensor_tensor
    tcp = nc.vector.tensor_copy
    ts_ = nc.vector.tensor_scalar
    stt = nc.vector.scalar_tensor_tensor

    # warmup source on the (idle) gpsimd queue so the PE starts ASAP
    nc.gpsimd.memset(wu_sb[:], 0.0)
    nc.vector.memset(halfpi_sb[:], PI / 2)
    nc.vector.memset(zerop_sb[:], 0.0)
    nc.vector.memset(zero1_sb[:], 0.0)
    nc.vector.memset(dum_sb[:], 0.0)
    nc.vector.memset(ones_sb[:], 1.0)
    nc.vector.memset(invdb_sb[:], 1.0 / D)

    # -------- DMA issues (sync + gpsimd only; scalar runs acts) --------
    # xpet/xpo columns are split across BOTH queues so the prefix chain
    # isn't gated by one queue's backlog.
    dma_s = nc.sync.dma_start
    dma_g = nc.gpsimd.dma_start
    dma_g(kwm_sb[:], io["kwm"])
    d_xt0 = dma_s(xt_sb[:, 0:512], io["xt"][:, 0:512])
    dma_g(vw_sb[:], io["vw"])
    d_xt1 = dma_s(xt_sb[:, 512:1024], io["xt"][:, 512:1024])
    dma_g(xpet_sb[:, 768:1152], io["xpetl"][:, 384:768])
    dma_s(xpet_sb[:, 0:384], io["xpetl"][:, 0:384])
    dma_g(xpet_sb[:, 1152:1536], io["xpeth"][:, 384:768])
    dma_s(xpet_sb[:, 384:768], io["xpeth"][:, 0:384])
    dma_g(xpo_sb[:, 768:1536], io["xpoh"])
    dma_s(xpo_sb[:, 0:768], io["xpol"])
    dma_g(cpk_sb[:], io["cpk"])
    dma_s(w1g_sb[:], io["w1g"])
    dma_g(xc_sb[:], io["xc"])
    dma_s(w2_sb[:], io["w2"])
    dma_g(cr1_sb[:], io["cr1"])
    dma_s(wog_sb[:], io["wog"])

    # ------ PE warmup / keep-warm fillers (HAM clock gate) ------
    def warm(n):
        t = pt.tile([128, 512], f32, tag="pt")
        for i in range(n):
            mm(t[:], wu_sb[:, 0:128], wu_sb[:, 128:640],
               start=True, stop=True)

    warm(5)

    # prime the silu act table (covers tanh/sin/square) exactly once
    a_dum = act(dum_sb[:], dum_sb[:], DUM_AF, bias=zero1_sb[:])

    # ---------------- own phases -> Q ----------------
    ph_ps = pa.tile([P, T], f32, tag="pa")
    for kt in range(2):
        mm(ph_ps[:], kwb_sb[:, 32 * kt: 32 * kt + 32],
           xt_sb[:, 512 * kt: 512 * kt + 512],
           start=(kt == 0), stop=(kt == 1))
    warm(4)
    a_tanh = act(tv_sb[P:2 * P, :], ph_ps[:], AF.Tanh, bias=zerop_sb[0:P, :])
    add_dep_helper(a_tanh.ins, a_dum.ins, sync=True,
                   reason="act table: silu table covers tanh/sin/square")
    ts_(out=tv_sb[0:P, :].bitcast(i32), in0=tv_sb[P:2 * P, :].bitcast(i32),
        scalar1=0x7FFFFFFF, scalar2=None, op0=OP.bitwise_and)
    # cos(pi*t) = sin(pi/2 - pi*|t|); sin table argument stays in range
    act(qb_sb[0:P, :], tv_sb[0:P, :], AF.Sin,
        bias=halfpi_sb[0:P, :], scale=-PI)
    a_oq = act(qb_sb[P:2 * P, :], tv_sb[P:2 * P, :], AF.Sin,
               bias=zerop_sb[0:P, :], scale=PI)

    # ---------------- V at odd own tokens ----------------
    for blk in range(2):
        vp = pa.tile([128, D], f32, tag="pa")
        for kt in range(2):
            mm(vp[:], xt_sb[:, 512 * kt + 256 * blk + 1:
                            512 * kt + 256 * blk + 256: 2],
               vw_sb[:, 256 * kt: 256 * kt + 256],
               start=(kt == 0), stop=(kt == 1))
        tcp(vodd_sb[:, 256 * blk: 256 * blk + 256], vp[:])

    warm(2)
    # ---------------- scores (odd tk only) + causal mask ----------------
    sc0 = pb.tile([128, 512], f32, tag="pb")
    mm(sc0[:], qb_sb[:, 0:255:2], qb_sb[:], start=True, stop=True)
    tt_(ss_sb[:, 0:256], sc0[:, 0:256], m0_sb[:], OP.mult)
    act(ss_sb[:, 256:512], sc0[:, 256:512], AF.Identity,
        bias=zerop_sb[:])
    sc1 = pa.tile([128, 256], f32, tag="pa")
    mm(sc1[:], qb_sb[:, 256:511:2], qb_sb[:, 256:512], start=True, stop=True)
    tt_(ss_sb[:, 512:768], sc1[:], m0_sb[:], OP.mult)

    # ---------------- prefix phases (token-major) -> kpre --------------
    pp_ps = pa.tile([128, 192], f32, tag="pa")
    for j in range(6):
        for kt in range(2):
            mm(pp_ps[:, 32 * j: 32 * j + 32],
               xpet_sb[:, 768 * kt + 128 * j: 768 * kt + 128 * j + 128],
               kwb_sb[:, 32 * kt: 32 * kt + 32],
               start=(kt == 0), stop=(kt == 1))
    a_pt = act(tvp_sb[:, 0:192], pp_ps[:], AF.Tanh, bias=zerop_sb[:])
    add_dep_helper(a_pt.ins, a_oq.ins, sync=True,
                   reason="own qb before prefix acts on the scalar queue")
    ts_(out=tvp_sb[:, 192:384].bitcast(i32), in0=tvp_sb[:, 0:192].bitcast(i32),
        scalar1=0x7FFFFFFF, scalar2=None, op0=OP.bitwise_and)
    act(kcs_sb[:, 0:192], tvp_sb[:, 192:384], AF.Sin,
        bias=halfpi_sb[:], scale=-PI)
    act(kcs_sb[:, 192:384], tvp_sb[:, 0:192], AF.Sin,
        bias=zerop_sb[:], scale=PI)

    warm(4)
    # ---------------- prefix G and S ----------------
    # cos/sin accumulation groups live in different PSUM banks (pt vs pa
    # pools) so they can interleave while sharing the xpo stationary.
    for dh in range(2):
        gpc = pt.tile([128, 32], f32, tag="pt")
        gps = pa.tile([128, 32], f32, tag="pa")
        for j in range(6):
            mm(gpc[:],
               xpo_sb[:, 256 * j + 128 * dh: 256 * j + 128 * dh + 128],
               kcs_sb[:, 32 * j: 32 * j + 32],
               start=(j == 0), stop=(j == 5))
            mm(gps[:],
               xpo_sb[:, 256 * j + 128 * dh: 256 * j + 128 * dh + 128],
               kcs_sb[:, 192 + 32 * j: 192 + 32 * j + 32],
               start=(j == 0), stop=(j == 5))
        tcp(gT_sb[:, 64 * dh: 64 * dh + 32], gpc[:])
        tcp(gT_sb[:, 64 * dh + 32: 64 * dh + 64], gps[:])
    s_ps = pa.tile([2 * P, D], f32, tag="pa")
    for kt in range(2):
        mm(s_ps[:], gT_sb[:, 64 * kt: 64 * kt + 64],
           vw_sb[:, 256 * kt: 256 * kt + 256],
           start=(kt == 0), stop=(kt == 1))
    act(s_sb[:], s_ps[:], AF.Identity, bias=zerop_sb[0:2 * P, :])

    # ------------- retrieved^T = tril(V^T ss) + S^T Q -------------
    retr = []
    for dh in range(2):
        rp = pb.tile([128, 512], f32, tag="pb")
        # one PSUM bank allows a single open accumulation group: close the
        # first-half group before opening the second-half one
        mm(rp[:, 0:256], vodd_sb[:, 128 * dh: 128 * dh + 128],
           ss_sb[:, 0:256], start=True, stop=False)
        mm(rp[:, 0:256], s_sb[:, 128 * dh: 128 * dh + 128],
           qb_sb[:, 0:256], start=False, stop=True)
        mm(rp[:, 256:512], vodd_sb[:, 128 * dh: 128 * dh + 128],
           ss_sb[:, 256:512], start=True, stop=False)
        mm(rp[:, 256:512], vodd_sb[:, 256 + 128 * dh: 256 + 128 * dh + 128],
           ss_sb[:, 512:768], start=False, stop=False)
        mm(rp[:, 256:512], s_sb[:, 128 * dh: 128 * dh + 128],
           qb_sb[:, 256:512], start=False, stop=True)
        retr.append(rp)

    warm(3)
    # ------- LN1 stats: casts/squares split over DVE and scalar -------
    tcp(r_sb[:, 0:512], retr[0][:])                       # DVE cast
    a_c1 = act(r_sb[:, 512:1024], retr[1][:], AF.Identity,
               bias=zerop_sb[:])                          # scalar copy
    act(sq_sb[:, 0:512], retr[0][:], AF.Square, bias=zerop_sb[:])
    tt_(sq_sb[:, 512:1024], r_sb[:, 512:1024], r_sb[:, 512:1024], OP.mult)
    st1 = pa.tile([1, T], f32, tag="pa")
    ms1 = pa.tile([1, T], f32, tag="pa2", bufs=1)
    for kt in range(2):
        mm(st1[0:1, :], invdb_sb[:, 0:1], r_sb[:, 512 * kt: 512 * kt + 512],
           start=(kt == 0), stop=(kt == 1))
    for kt in range(2):
        mm(ms1[0:1, :], invdb_sb[:, 0:1], sq_sb[:, 512 * kt: 512 * kt + 512],
           start=(kt == 0), stop=(kt == 1))
    act(mean1_sb[:], st1[0:1, :], AF.Identity, bias=zerop_sb[0:1, :])
    tt_(m2_sb[:], mean1_sb[:], mean1_sb[:], OP.mult)
    stt(out=var1_sb[:], in0=ms1[0:1, :], scalar=1e-5, in1=m2_sb[:],
        op0=OP.add, op1=OP.subtract)

    def row_rstd(var_sb, std_sb, rstd_ap):
        # rstd row = 1/sqrt(var): Sqrt on the scalar engine (its own act
        # table load hides in an idle window), fast-reciprocal on DVE.
        act(std_sb[:], var_sb[:], AF.Sqrt, bias=zero1_sb[:])
        nc.vector.reciprocal_approx_fast(out=rstd_ap, in_=std_sb[:])

    # ---- W1 on raw r while the rstd chain runs on DVE (post-scale):
    #   (W1g^T r + c1n (x) mean1) * rstd1 == W1g^T(LN1(r))
    def w1_block(m):
        hp = pb.tile([128, 512], f32, tag="pb")
        for kt in range(2):
            mm(hp[:], w1g_sb[:, 512 * kt + 128 * m: 512 * kt + 128 * m + 128],
               r_sb[:, 512 * kt: 512 * kt + 512],
               start=(kt == 0), stop=False)
        mm(hp[:], c1n_sb[0:1, 128 * m: 128 * m + 128], mean1_sb[:],
           start=False, stop=True)
        return hp

    hps = [w1_block(0)]
    row_rstd(var1_sb, std1_sb, rstd1_sb[:])
    tcp(rstd1h_sb[:], rstd1_sb[:])
    hps.append(w1_block(1))
    hps.append(w1_block(2))
    rb1 = pa.tile([128, 512], f32, tag="pa")
    mm(rb1[:], ones_sb[:], rstd1h_sb[:], start=True, stop=True)
    tcp(rb1s_sb[:], rb1[:])
    hps.append(w1_block(3))
    for m in range(4):
        tt_(rs_sb[:, 512 * m: 512 * m + 512],
            hps[m][:], rb1s_sb[:], OP.mult)
        act(h_sb[:, 512 * m: 512 * m + 512],
            rs_sb[:, 512 * m: 512 * m + 512], GELU_AF,
            bias=cb1_sb[:, m: m + 1])

    warm(10)
    # ---------------- W2 -> refined ----------------
    ref = []
    for dh in range(2):
        fp = pb.tile([128, 512], f32, tag="pb")
        for kt in range(4):
            mm(fp[:], w2_sb[:, 256 * kt + 128 * dh: 256 * kt + 128 * dh + 128],
               h_sb[:, 512 * kt: 512 * kt + 512],
               start=(kt == 0), stop=(kt == 3))
        ref.append(fp)

    # ------- LN2 stats (+b2 bias), split over DVE and scalar -------
    warm(3)
    ts_(out=f_sb[:, 0:512], in0=ref[0][:], scalar1=b2c_sb[:, 0:1],
        scalar2=None, op0=OP.add)                          # DVE
    act(f_sb[:, 512:1024], ref[1][:], AF.Identity, bias=b2c_sb[:, 1:2])
    act(sq2_sb[:, 0:512], ref[0][:], AF.Square, bias=b2c_sb[:, 0:1])
    tt_(sq2_sb[:, 512:1024], f_sb[:, 512:1024], f_sb[:, 512:1024], OP.mult)
    st2 = pa.tile([1, T], f32, tag="pa")
    ms2 = pa.tile([1, T], f32, tag="pa2", bufs=1)
    for kt in range(2):
        mm(st2[0:1, :], invdb_sb[:, 0:1], f_sb[:, 512 * kt: 512 * kt + 512],
           start=(kt == 0), stop=(kt == 1))
    for kt in range(2):
        mm(ms2[0:1, :], invdb_sb[:, 0:1], sq2_sb[:, 512 * kt: 512 * kt + 512],
           start=(kt == 0), stop=(kt == 1))
    act(mean2_sb[:], st2[0:1, :], AF.Identity, bias=zerop_sb[0:1, :])
    tt_(m2b_sb[:], mean2_sb[:], mean2_sb[:], OP.mult)
    stt(out=var2_sb[:], in0=ms2[0:1, :], scalar=1e-5, in1=m2b_sb[:],
        op0=OP.add, op1=OP.subtract)

    # Wo (token-major) + rank-1 mean fix; Newton for rstd2 runs on DVE
    # underneath these matmuls; rstd2 applies inside the residual op.
    ops = []
    for tm in range(4):
        op = pa.tile([128, D], f32, tag="pa")
        for dh in range(2):
            mm(op[:], f_sb[:, 512 * dh + 128 * tm: 512 * dh + 128 * tm + 128],
               wog_sb[:, 256 * dh: 256 * dh + 256],
               start=(dh == 0), stop=False)
        mm(op[:], mean2_sb[0:1, 128 * tm: 128 * tm + 128], c1on_sb[:],
           start=False, stop=True)
        ops.append(op)
        if tm == 0:
            # column-Newton rsqrt on DVE (keeps the sqrt act table
            # unloaded here -> only 3 table loads total); overlapped
            # under the Wo matmuls.
            vc2 = pt.tile([128, 4], f32, tag="pt")
            for j in range(4):
                nc.tensor.transpose(vc2[:, j: j + 1],
                                    var2_sb[0:1, 128 * j: 128 * j + 128],
                                    eyef_sb[0:1, 0:1])
            tcp(stdc2_sb[:], vc2[:])
            ts_(out=rstdc2_sb[:].bitcast(i32), in0=stdc2_sb[:].bitcast(i32),
                scalar1=1, scalar2=None, op0=OP.logical_shift_right)
            ts_(out=rstdc2_sb[:].bitcast(i32), in0=rstdc2_sb[:].bitcast(i32),
                scalar1=-1, scalar2=0x5F3759DF, op0=OP.mult, op1=OP.add)
            tt_(nwt2_sb[:], rstdc2_sb[:], rstdc2_sb[:], OP.mult)
            stt(out=nwt2_sb[:], in0=nwt2_sb[:], scalar=-0.5, in1=stdc2_sb[:],
                op0=OP.mult, op1=OP.mult)
            stt(out=rstdc2_sb[:], in0=nwt2_sb[:], scalar=1.5, in1=rstdc2_sb[:],
                op0=OP.add, op1=OP.mult)
        if tm >= 1:
            t0 = tm - 1
            stt(out=out_sb[:, 256 * t0: 256 * t0 + 256], in0=ops[t0][:],
                scalar=rstdc2_sb[:, t0: t0 + 1],
                in1=xc_sb[:, 256 * t0: 256 * t0 + 256],
                op0=OP.mult, op1=OP.add)
            dq = dma_s if t0 % 2 == 0 else dma_g
            dq(outc[128 * t0: 128 * t0 + 128, :],
               out_sb[:, 256 * t0: 256 * t0 + 256])
    stt(out=out_sb[:, 768:1024], in0=ops[3][:],
        scalar=rstdc2_sb[:, 3:4], in1=xc_sb[:, 768:1024],
        op0=OP.mult, op1=OP.add)
    dma_g(outc[384:512, :], out_sb[:, 768:1024])

    pb.release()
    pa.release()
    pt.release()
    sb.release()


_CACHE = {}


def _get_nc():
    if "nc" not in _CACHE:
        _CACHE["nc"] = _build()
    return _CACHE["nc"]


def _bf(a):
    return np.asarray(a, np.float32).astype(ml_dtypes.bfloat16)


def _f16(a):
    return np.asarray(a, np.float32).astype(np.float16)


def _img(a, nblk, w):
    # [nblk*128, w] -> [128, nblk*w] SBUF image (block b at cols w*b)
    a = np.asarray(a, np.float32)
    return np.ascontiguousarray(
        a.reshape(nblk, 128, w).transpose(1, 0, 2).reshape(128, nblk * w))


def kernel(**inputs):
    x = np.asarray(inputs["x"], np.float32)
    key_W = np.asarray(inputs["key_W"], np.float32)
    key_b = np.asarray(inputs["key_b"], np.float32)
    val_W = np.asarray(inputs["val_W"], np.float32)
    val_b = np.asarray(inputs["val_b"], np.float32)
    ln1_g = np.asarray(inputs["ln1_g"], np.float32)
    ln1_b = np.asarray(inputs["ln1_b"], np.float32)
    W1 = np.asarray(inputs["W1"], np.float32)
    b1 = np.asarray(inputs["b1"], np.float32)
    W2 = np.asarray(inputs["W2"], np.float32)
    b2 = np.asarray(inputs["b2"], np.float32)
    ln2_g = np.asarray(inputs["ln2_g"], np.float32)
    ln2_b = np.asarray(inputs["ln2_b"], np.float32)
    Wo = np.asarray(inputs["Wo"], np.float32)
    bo = np.asarray(inputs["bo"], np.float32)

    # these are identically zero for this module; the kernel folds them out
    assert np.allclose(val_b, 0.0), "nonzero val_b unsupported"
    assert np.allclose(key_b, 0.0), "nonzero key_b unsupported"
    assert np.allclose(bo + ln2_b @ Wo, 0.0), "nonzero output bias unsupported"

    w1g = ln1_g[:, None] * W1
    wog = ln2_g[:, None] * Wo
    m0 = (np.arange(1, 256, 2)[:, None] <=
          np.arange(256)[None, :]).astype(np.float32)
    kwm = np.concatenate([_img(key_W, 2, P), m0], axis=1)
    cpk = np.concatenate([
        np.eye(128, dtype=np.float32),
        (b1 + ln1_b @ W1).reshape(128, 4, order="F"),
        b2.reshape(128, 2, order="F")], axis=1).astype(np.float32)
    cr1 = np.concatenate([-w1g.sum(0), -wog.sum(0)])[None, :]
    shared = {
        "kwm": _f16(kwm),
        "vw": _f16(_img(val_W, 2, D)),
        "w1g": _bf(_img(w1g, 2, H)),
        "w2": _bf(_img(W2, 4, D)),
        "wog": _bf(_img(wog, 2, D)),
        "cpk": np.ascontiguousarray(cpk), "cr1": _bf(cr1),
    }
    in_maps = []
    for i in range(N_CORES):
        b, c = divmod(i, 4)
        l0 = c * T
        npairs = l0 // 2
        xpe = np.zeros((PRE, D), np.float32)
        xpo = np.zeros((PRE, D), np.float32)
        if npairs:
            xpe[:npairs] = x[b, 0:l0 - 1:2]
            xpo[:npairs] = x[b, 1:l0:2]
        xch = np.ascontiguousarray(x[b, l0:l0 + T])
        xpet_img = _img(np.ascontiguousarray(xpe.T), 2, PRE)
        xpo_img = _img(xpo, 6, D)
        in_maps.append({
            "xt": _f16(_img(xch.T, 2, T)),
            "xc": _f16(_img(xch, 4, D)),
            "xpetl": _f16(np.concatenate(
                [xpet_img[:, 0:384], xpet_img[:, 768:1152]], axis=1)),
            "xpeth": _f16(np.concatenate(
                [xpet_img[:, 384:768], xpet_img[:, 1152:1536]], axis=1)),
            "xpol": _f16(xpo_img[:, 0:768]),
            "xpoh": _f16(xpo_img[:, 768:1536]),
            **shared,
        })

    nc = _get_nc()
    res = run_bass_kernel_spmd(nc, in_maps, core_ids=list(range(N_CORES)),
                               **_CACHE.get("run_kwargs", {}))
    _CACHE["last_result"] = res
    out = np.empty((B, L, D), np.float32)
    for i in range(N_CORES):
        b, c = divmod(i, 4)
        out[b, c * T:(c + 1) * T] = res.results[i]["outc"]
    return out
